# revision 50
# baseline (speedup 1.0000x reference)
"""Trainium2 Bass kernel for nn_Attn_33028298506245 (sparse MLA-style attention).

Sharding: tensor-parallel over the 16 heads -> 2 heads per NeuronCore (8 cores).
Shared work (compressed q/kv projection, rmsnorm stats, gate, token top-k
selection gather) is replicated per core; the final out-projection is computed
per-core on that core's head slice and the partial [T, C] outputs are summed on
the host (the all-reduce / unshard step).

Device algorithm (per core, transposed "d-on-partitions" orientation):
  - qkv_nT = [w_cq|w_ckv]^T x^T               [128, T]  (rows 0:96 q, 96:128 kv)
  - rms stats via ones-mask matmul; inv scales folded into Q^T/K^T columns
  - Q^T/K^T produced directly transposed; RoPE applied in [d, T] layout
  - attention computed as S^T = K^T_block^T Q^T  [keys, queries]; softmax along
    the key (partition) axis via exp + ones-row-augmented V matmul (denominator
    comes out as row 96 of the PV accumulation); causal masking via a
    precomputed staircase strip multiply on the 4 diagonal key-blocks
  - branch 2 uses dma_gather (transpose=True) to build sel^T from the top-512
    tokens; rope positions are 0..511 as in the reference
  - gated combine of the 3 branches, then out-proj on the head slice
"""
import os
import sys

for _p in ("/opt/trn_rl_repo", "/root/.axon_site/_ro/trn_rl_repo"):
    if os.path.isdir(_p) and _p not in sys.path:
        sys.path.append(_p)

import numpy as np
import ml_dtypes

import concourse.bass as bass
import concourse.mybir as mybir
import concourse.tile as tile
from concourse import bacc
from concourse import bass_utils
from concourse.masks import make_identity

bf16 = ml_dtypes.bfloat16
FP32 = mybir.dt.float32
BF16 = mybir.dt.bfloat16

B, T, C = 1, 2048, 1024
H = 16
NOPE, ROPE_D, VHD = 32, 64, 96
KV_RANK, Q_RANK = 32, 96
S_KEEP = 512
EPS = 1e-6
HEAD_D = NOPE + ROPE_D          # 96
SCALE = 1.0 / float(np.sqrt(HEAD_D))
N_CORES = 8
NS = 4                          # strips of 512 queries
SW = 512                        # strip width
TB = 16                         # 128-token blocks
CB = 8                          # 128-channel blocks
VA = VHD + 1                    # V augmented with ones row -> denominator

AF = mybir.ActivationFunctionType
OP = mybir.AluOpType


def _emit(nc):
    dt_in = {}

    def din(name, shape, dtype):
        t = nc.dram_tensor(name, shape, dtype, kind="ExternalInput")
        dt_in[name] = t
        return t

    xT_d = din("xT", [C, T], BF16)
    xbf_d = din("x_bf", [T, C], BF16)
    cos_d = din("cos_t", [32, T], BF16)
    sin_d = din("sin_t", [32, T], BF16)
    mask_d = din("mask_strip", [128, 128], BF16)
    idx_d = din("idx16", [128, S_KEEP // 16], mybir.dt.int16)
    wqkv_d = din("Wqkv", [C, 128], BF16)
    wdq_d = din("Wdq", [Q_RANK, 192], BF16)
    wdk_d = din("Wdk", [KV_RANK, 192], BF16)
    wdv_d = din("Wdv", [KV_RANK, 192], BF16)
    wwk_d = din("Wwk", [C, 192], BF16)
    wsk_d = din("Wsk", [C, 192], BF16)
    wwvgi_d = din("Wwv_gi", [C, 196], BF16)
    wsv_d = din("Wsv", [C, 192], BF16)
    wproj_d = din("Wproj", [Q_RANK, 2 * C], BF16)
    bgate_d = din("bgate", [1, 3], FP32)
    y_d = nc.dram_tensor("y", [T, C], FP32, kind="ExternalOutput")

    with tile.TileContext(nc) as tc:
        _body(nc, tc, xT_d, xbf_d, cos_d, sin_d, mask_d, idx_d, wqkv_d, wdq_d,
              wdk_d, wdv_d, wwk_d, wsk_d, wwvgi_d, wsv_d, wproj_d, bgate_d, y_d)
    return dt_in


def _body(nc, tc, xT_d, xbf_d, cos_d, sin_d, mask_d, idx_d, wqkv_d, wdq_d,
          wdk_d, wdv_d, wwk_d, wsk_d, wwvgi_d, wsv_d, wproj_d, bgate_d, y_d):
    from contextlib import ExitStack
    ctx = ExitStack()
    with ctx:
        const = ctx.enter_context(tc.tile_pool(name="const", bufs=1))
        sbA = ctx.enter_context(tc.tile_pool(name="sbA", bufs=1))
        work = ctx.enter_context(tc.tile_pool(name="work", bufs=2))
        ropep = ctx.enter_context(tc.tile_pool(name="ropep", bufs=3))
        expp = ctx.enter_context(tc.tile_pool(name="expp", bufs=4))
        bcp = ctx.enter_context(tc.tile_pool(name="bcp", bufs=2))
        recp = ctx.enter_context(tc.tile_pool(name="recp", bufs=4))
        otp = ctx.enter_context(tc.tile_pool(name="otp", bufs=2))
        ysb = ctx.enter_context(tc.tile_pool(name="ysb", bufs=2))
        psS = ctx.enter_context(tc.tile_pool(name="psS", bufs=2, space="PSUM"))
        psO = ctx.enter_context(tc.tile_pool(name="psO", bufs=2, space="PSUM"))
        psP = ctx.enter_context(tc.tile_pool(name="psP", bufs=2, space="PSUM"))

        # ---- constants / weights into SBUF ----
        xT_sb = const.tile([128, CB, T], BF16)
        for cb in range(CB):
            nc.sync.dma_start(xT_sb[:, cb, :], xT_d.ap()[cb * 128:(cb + 1) * 128, :])
        cos_sb = const.tile([32, T], FP32)
        sin_sb = const.tile([32, T], FP32)
        nc.sync.dma_start(cos_sb, cos_d.ap())
        nc.sync.dma_start(sin_sb, sin_d.ap())
        mask_sb = const.tile([128, 128], BF16)
        nc.sync.dma_start(mask_sb, mask_d.ap())
        idx_sb = const.tile([128, S_KEEP // 16], mybir.dt.int16)
        nc.sync.dma_start(idx_sb, idx_d.ap())

        wqkv_sb = const.tile([128, CB, 128], BF16)
        nc.sync.dma_start(wqkv_sb, wqkv_d.ap().rearrange("(cb p) m -> p cb m", p=128))
        wdq_sb = const.tile([Q_RANK, 192], BF16)
        nc.sync.dma_start(wdq_sb, wdq_d.ap())
        wdk_sb = const.tile([KV_RANK, 192], BF16)
        nc.sync.dma_start(wdk_sb, wdk_d.ap())
        wdv_sb = const.tile([KV_RANK, 192], BF16)
        nc.sync.dma_start(wdv_sb, wdv_d.ap())
        wwk_sb = const.tile([128, CB, 192], BF16)
        nc.sync.dma_start(wwk_sb, wwk_d.ap().rearrange("(cb p) m -> p cb m", p=128))
        wsk_sb = const.tile([128, CB, 192], BF16)
        nc.sync.dma_start(wsk_sb, wsk_d.ap().rearrange("(cb p) m -> p cb m", p=128))
        wwvgi_sb = const.tile([128, CB, 196], BF16)
        nc.sync.dma_start(wwvgi_sb, wwvgi_d.ap().rearrange("(cb p) m -> p cb m", p=128))
        wsv_sb = const.tile([128, CB, 192], BF16)
        nc.sync.dma_start(wsv_sb, wsv_d.ap().rearrange("(cb p) m -> p cb m", p=128))
        wproj_sb = const.tile([Q_RANK, 2 * C], BF16)
        nc.sync.dma_start(wproj_sb, wproj_d.ap())
        bgate_sb = const.tile([1, 3], FP32)
        nc.sync.dma_start(bgate_sb, bgate_d.ap())

        ones96_bf = const.tile([Q_RANK, 1], BF16)
        nc.vector.memset(ones96_bf, 1.0)
        ones32_bf = const.tile([KV_RANK, 1], BF16)
        nc.vector.memset(ones32_bf, 1.0)
        ones128_f32 = const.tile([128, 1], FP32)
        nc.vector.memset(ones128_f32, 1.0)
        ident1 = const.tile([1, 1], FP32)
        nc.vector.memset(ident1, 1.0)

        # ---- persistent intermediates ----
        qnT = sbA.tile([Q_RANK, T], BF16)               # q_nT (unnormalized)
        kvT = sbA.tile([KV_RANK, T], BF16)              # kv_nT (unnormalized)
        selT = sbA.tile([128, CB, S_KEEP], BF16)        # sel^T gathered
        inv_q_rows = [sbA.tile([1, SW], FP32, tag=f"invq{s}", name=f"invq{s}") for s in range(NS)]
        inv_kv_rows = [sbA.tile([1, SW], FP32, tag=f"invk{s}", name=f"invk{s}") for s in range(NS)]
        inv_colT = sbA.tile([128, TB], FP32)            # per-token kv inv (columns)
        gi_acc = sbA.tile([128, 4], FP32)
        gate_sb = sbA.tile([1, 3], FP32)
        QT = [sbA.tile([Q_RANK, T], BF16, tag=f"QT{h}", name=f"QT{h}") for h in range(2)]
        K1T = [sbA.tile([Q_RANK, T], BF16, tag=f"K1T{h}", name=f"K1T{h}") for h in range(2)]
        KwT = [sbA.tile([Q_RANK, T], BF16, tag=f"KwT{h}", name=f"KwT{h}") for h in range(2)]
        KsT = [sbA.tile([Q_RANK, S_KEEP], BF16, tag=f"KsT{h}", name=f"KsT{h}") for h in range(2)]
        V1 = [sbA.tile([128, TB, VA], BF16, tag=f"V1{h}", name=f"V1_{h}") for h in range(2)]
        Vw = [sbA.tile([128, TB, VA], BF16, tag=f"Vw{h}", name=f"Vw_{h}") for h in range(2)]
        Vs = [sbA.tile([128, 4, VA], BF16, tag=f"Vs{h}", name=f"Vs_{h}") for h in range(2)]
        OTt = {(h, s): sbA.tile([Q_RANK, SW], BF16, tag=f"OTt{h}_{s}", name=f"OTt{h}_{s}")
               for h in range(2) for s in range(NS)}

        # ones rows of augmented V (never overwritten afterwards)
        for h in range(2):
            nc.vector.memset(V1[h][:, :, VHD:VA], 1.0)
            nc.vector.memset(Vw[h][:, :, VHD:VA], 1.0)
            nc.vector.memset(Vs[h][:, :, VHD:VA], 1.0)
        nc.vector.memset(gi_acc, 0.0)

        # ---- A: shared projection + rms stats per strip ----
        for s in range(NS):
            t0 = s * SW
            ps = psP.tile([128, SW], FP32, tag="pp")
            for cb in range(CB):
                nc.tensor.matmul(ps, wqkv_sb[:, cb, :], xT_sb[:, cb, t0:t0 + SW],
                                 start=(cb == 0), stop=(cb == CB - 1))
            nc.scalar.copy(qnT[:, t0:t0 + SW], ps[0:Q_RANK, :])
            # 32-partition quadrant move (q3 -> q0) on DVE
            nc.vector.tensor_copy(kvT[:, t0:t0 + SW], ps[Q_RANK:128, :])
            qsq = work.tile([Q_RANK, SW], BF16, tag="qsq")
            nc.vector.tensor_mul(qsq, qnT[:, t0:t0 + SW], qnT[:, t0:t0 + SW])
            ksq = work.tile([KV_RANK, SW], BF16, tag="ksq")
            nc.vector.tensor_mul(ksq, kvT[:, t0:t0 + SW], kvT[:, t0:t0 + SW])
            for (onev, sqt, invr, rk) in ((ones96_bf, qsq, inv_q_rows[s], Q_RANK),
                                          (ones32_bf, ksq, inv_kv_rows[s], KV_RANK)):
                ssq = psO.tile([1, SW], FP32, tag="ot")
                nc.tensor.matmul(ssq, onev, sqt, start=True, stop=True)
                mtmp = work.tile([1, SW], FP32, tag="mtmp")
                nc.vector.tensor_scalar(mtmp, ssq, 1.0 / rk, EPS,
                                        op0=OP.mult, op1=OP.add)
                stmp = work.tile([1, SW], FP32, tag="stmp")
                nc.scalar.activation(stmp, mtmp, AF.Sqrt)
                nc.vector.reciprocal(invr, stmp)
            # transpose kv inv to column form for V1 scaling
            for b in range(4):
                tp = psP.tile([128, 1], FP32, tag="pp")
                nc.tensor.transpose(tp, inv_kv_rows[s][:, b * 128:(b + 1) * 128],
                                    ident1)
                nc.scalar.copy(inv_colT[:, s * 4 + b:s * 4 + b + 1], tp)

        # ---- Vwin + gate/imp projection (heads paired) ----
        for b in range(TB):
            ps = psP.tile([128, 196], FP32, tag="pp")
            for cb in range(CB):
                nc.tensor.matmul(ps, xT_sb[:, cb, b * 128:(b + 1) * 128],
                                 wwvgi_sb[:, cb, :],
                                 start=(cb == 0), stop=(cb == CB - 1))
            for h in range(2):
                nc.scalar.copy(Vw[h][:, b, 0:VHD], ps[:, h * 96:h * 96 + 96])
            nc.vector.tensor_add(gi_acc, gi_acc, ps[:, 192:196])

        # gate = softmax(sum/T + b_gate)
        glp = psO.tile([1, 4], FP32, tag="ot")
        nc.tensor.matmul(glp, ones128_f32, gi_acc, start=True, stop=True)
        gl = work.tile([1, 4], FP32, tag="gl")
        nc.scalar.activation(gl, glp, AF.Copy, scale=1.0 / T)
        nc.vector.tensor_add(gl[0:1, 0:3], gl[0:1, 0:3], bgate_sb)
        ge = work.tile([1, 3], FP32, tag="ge")
        nc.scalar.activation(ge, gl[0:1, 0:3], AF.Exp)
        gs = work.tile([1, 1], FP32, tag="gs")
        nc.vector.reduce_sum(gs, ge, axis=mybir.AxisListType.X)
        gr = work.tile([1, 1], FP32, tag="gr")
        nc.vector.reciprocal(gr, gs)
        nc.vector.tensor_scalar_mul(gate_sb, ge, gr)

        # ---- top-k gather: selT[p, cb, i] = x_bf[idx[i], cb*128+p] ----
        nc.gpsimd.dma_gather(
            out_ap=selT[:],
            in_ap=xbf_d.ap(),
            idxs_ap=idx_sb[:],
            num_idxs=S_KEEP,
            num_idxs_reg=S_KEEP,
            elem_size=C,
            transpose=True,
        )

        # ---- Vsel (heads paired) ----
        for b in range(4):
            ps = psP.tile([128, 192], FP32, tag="pp")
            for cb in range(CB):
                nc.tensor.matmul(ps, selT[:, cb, b * 128:(b + 1) * 128],
                                 wsv_sb[:, cb, :],
                                 start=(cb == 0), stop=(cb == CB - 1))
            for h in range(2):
                nc.scalar.copy(Vs[h][:, b, 0:VHD], ps[:, h * 96:h * 96 + 96])

        # ---- V1 (heads paired, kv inv scaling at eviction) ----
        for b in range(TB):
            ps = psP.tile([128, 192], FP32, tag="pp")
            nc.tensor.matmul(ps, kvT[:, b * 128:(b + 1) * 128],
                             wdv_sb, start=True, stop=True)
            for h in range(2):
                nc.vector.tensor_scalar_mul(V1[h][:, b, 0:VHD],
                                            ps[:, h * 96:h * 96 + 96],
                                            inv_colT[:, b:b + 1])

        def rope_from(ps, out_t, t0, cos_slc, sin_slc, inv_bc):
            """Evict [96, w] psum -> out_t cols t0:t0+w applying optional
            per-column inv scaling (inv_bc broadcast tile or None) + RoPE on
            rows 32:96."""
            w = cos_slc.shape[-1]
            if inv_bc is not None:
                nc.vector.tensor_mul(out_t[0:32, t0:t0 + w], ps[0:32, :],
                                     inv_bc[0:32, :])
                p1 = ropep.tile([32, SW], FP32, tag="p1")
                p2 = ropep.tile([32, SW], FP32, tag="p2")
                nc.vector.tensor_mul(p1[:, 0:w], ps[32:64, :], inv_bc[32:64, :])
                nc.vector.tensor_mul(p2[:, 0:w], ps[64:96, :], inv_bc[64:96, :])
                r, i = p1[:, 0:w], p2[:, 0:w]
            else:
                nc.scalar.copy(out_t[0:32, t0:t0 + w], ps[0:32, :])
                r, i = ps[32:64, :], ps[64:96, :]
            ta = ropep.tile([32, SW], FP32, tag="ta")
            tb = ropep.tile([32, SW], FP32, tag="tb")
            nc.vector.tensor_mul(ta[:, 0:w], r, cos_slc)
            nc.vector.tensor_mul(tb[:, 0:w], i, sin_slc)
            nc.vector.tensor_sub(out_t[32:64, t0:t0 + w], ta[:, 0:w], tb[:, 0:w])
            nc.vector.tensor_mul(ta[:, 0:w], r, sin_slc)
            nc.vector.tensor_mul(tb[:, 0:w], i, cos_slc)
            nc.vector.tensor_add(out_t[64:96, t0:t0 + w], ta[:, 0:w], tb[:, 0:w])

        # ---- per-head projections (both heads) ----
        def d_proj(h):
            sl = slice(h * 96, h * 96 + 96)
            for s2 in range(NS // 2):
                t0 = s2 * 2 * SW
                W2 = 2 * SW
                for (wt, src, out_t, ib) in (
                        (wdq_sb[:, sl], qnT, QT[h], ibq_f),
                        (wdk_sb[:, sl], kvT, K1T[h], ibk_f)):
                    stg = ropep.tile([Q_RANK, 2 * SW], BF16, tag="stg",
                                     name=f"stgd{h}{s2}")
                    for j in range(2):
                        ts_ = t0 + j * SW
                        ps = psP.tile([Q_RANK, SW], FP32, tag="pp",
                                      name=f"psd{h}{s2}{j}")
                        nc.tensor.matmul(ps, wt, src[:, ts_:ts_ + SW],
                                         start=True, stop=True)
                        nc.scalar.copy(stg[:, j * SW:(j + 1) * SW], ps)
                    rope_stg(stg, out_t, t0, W2, t0,
                             ib[:, t0:t0 + W2] if ib is not None else None)
                stg = ropep.tile([Q_RANK, 2 * SW], BF16, tag="stg", name=f"stgw{h}{s2}")
                for j in range(2):
                    ts_ = t0 + j * SW
                    ps = psP.tile([Q_RANK, SW], FP32, tag="pp", name=f"psw{h}{s2}{j}")
                    for cb in range(CB):
                        nc.tensor.matmul(ps, wwk_sb[:, cb, sl],
                                         xT_sb[:, cb, ts_:ts_ + SW],
                                         start=(cb == 0), stop=(cb == CB - 1))
                    nc.scalar.copy(stg[:, j * SW:(j + 1) * SW], ps)
                rope_stg(stg, KwT[h], t0, W2, t0, None)

            # Ksel^T (rope positions 0..S_KEEP-1)
            stg = ropep.tile([Q_RANK, 2 * SW], BF16, tag="stg", name=f"stgs{h}")
            ps = psP.tile([Q_RANK, S_KEEP], FP32, tag="pp", name=f"pss{h}")
            for cb in range(CB):
                nc.tensor.matmul(ps, wsk_sb[:, cb, sl], selT[:, cb, :],
                                 start=(cb == 0), stop=(cb == CB - 1))
            nc.scalar.copy(stg[:, 0:S_KEEP], ps)
            rope_stg(stg, KsT[h], 0, S_KEEP, 0, None)

        # ---- attention per head (projections emitted just before each) ----
        for h in range(2):
            d_proj(h)
            # attention: 3 branches per strip
            for s in range(NS):
                t0 = s * SW
                qs = QT[h][:, t0:t0 + SW]
                otacc = None
                for br, (KT, Vt, causal) in enumerate([
                        (K1T[h], V1[h], True),
                        (KsT[h], Vs[h], False),
                        (KwT[h], Vw[h], True)]):
                    nk = 4 * (s + 1) if causal else 4
                    otp_ps = psO.tile([VA, SW], FP32, tag="ot")
                    for pair in range(nk // 2):
                        # diagonal key-blocks only need columns >= c0 (keys can
                        # never attend to earlier queries); the rest is masked
                        def col0(kb):
                            return (max(0, kb * 128 - s * SW)
                                    if causal and kb >= nk - 4 else 0)
                        kbs = (pair * 2, pair * 2 + 1)
                        c0s = [col0(kb) for kb in kbs]
                        cu = min(c0s)
                        sp = psS.tile([128, 2, SW], FP32, tag="sp")
                        for j, kb in enumerate(kbs):
                            nc.tensor.matmul(sp[:, j, c0s[j]:SW],
                                             KT[:, kb * 128:(kb + 1) * 128],
                                             qs[:, c0s[j]:SW],
                                             start=True, stop=True)
                        es = expp.tile([128, 2, SW], BF16, tag="es")
                        if c0s[0] == c0s[1]:
                            nc.scalar.activation(es[:, :, cu:SW], sp[:, :, cu:SW],
                                                 AF.Exp, scale=SCALE)
                        else:
                            for j in range(2):
                                nc.scalar.activation(es[:, j, c0s[j]:SW],
                                                     sp[:, j, c0s[j]:SW],
                                                     AF.Exp, scale=SCALE)
                        for j, kb in enumerate(kbs):
                            c0 = c0s[j]
                            if causal and kb >= nk - 4:
                                # partial triangle lives in the first 128 cols
                                nc.vector.tensor_mul(es[:, j, c0:c0 + 128],
                                                     es[:, j, c0:c0 + 128],
                                                     mask_sb[:, 0:128])
                            nc.tensor.matmul(otp_ps[:, c0:SW], Vt[:, kb, :],
                                             es[:, j, c0:SW],
                                             start=(kb == 0), stop=(kb == nk - 1))
                    rec = recp.tile([1, SW], FP32, tag="rec")
                    nc.vector.reciprocal(rec, otp_ps[VHD:VA, :])
                    rsc = recp.tile([1, SW], FP32, tag="rsc")
                    nc.vector.tensor_scalar_mul(rsc, rec, gate_sb[0:1, br:br + 1])
                    obc = bcp.tile([96, SW], FP32, tag="obc")
                    nc.gpsimd.partition_broadcast(obc, rsc)
                    if br == 0:
                        otacc = otp.tile([96, SW], FP32, tag="otacc")
                        nc.vector.tensor_mul(otacc, otp_ps[0:VHD, :], obc)
                    else:
                        tmp = otp.tile([96, SW], FP32, tag="otmp")
                        nc.vector.tensor_mul(tmp, otp_ps[0:VHD, :], obc)
                        dst = OTt[(h, s)] if br == 2 else otacc
                        nc.vector.tensor_add(dst, otacc, tmp)

        # ---- out-projection (partial y on the 2-head slice) ----
        for b in range(TB):
            s = b // 4
            c0 = (b % 4) * 128
            yt = ysb.tile([128, C], FP32, tag="yt")
            for half in range(2):
                yp = psP.tile([128, SW], FP32, tag="pp")
                for h in range(2):
                    nc.tensor.matmul(yp, OTt[(h, s)][:, c0:c0 + 128],
                                     wproj_sb[:, h * C + half * SW:
                                              h * C + half * SW + SW],
                                     start=(h == 0), stop=(h == 1))
                nc.vector.tensor_copy(yt[:, half * SW:half * SW + SW], yp)
            nc.sync.dma_start(y_d.ap()[b * 128:(b + 1) * 128, :], yt)


# ------------------------------------------------------------------
# host side
# ------------------------------------------------------------------

_BUILT = None


def _build():
    global _BUILT
    if _BUILT is None:
        nc = bacc.Bacc(None, target_bir_lowering=False, debug=False)
        _emit(nc)
        nc.finalize()
        _BUILT = nc
    return _BUILT


def _bf(x):
    return np.ascontiguousarray(np.asarray(x, np.float32)).astype(bf16)


def host_prep(inputs):
    inp = {k: np.asarray(v, np.float32) if np.asarray(v).dtype == np.float32
           else np.asarray(v) for k, v in inputs.items()}
    x = np.asarray(inp['x'], np.float32)[0]          # [T, C]
    shared = {}
    shared['xT'] = _bf(x.T)
    shared['x_bf'] = _bf(x)
    inv = 1.0 / (10000.0 ** (np.arange(0, ROPE_D, 2, dtype=np.float32) / ROPE_D))
    f = np.outer(inv, np.arange(T, dtype=np.float32))
    shared['cos_t'] = np.cos(f).astype(bf16)
    shared['sin_t'] = np.sin(f).astype(bf16)
    k = np.arange(128)[:, None]
    cgrid = np.arange(128)[None, :]
    shared['mask_strip'] = (k <= cgrid).astype(bf16)
    scores = x @ np.asarray(inp['w_imp'], np.float32)[:, 0]
    idx = np.sort(np.argsort(-scores)[:S_KEEP])
    wrapped = idx.astype(np.int16).reshape(S_KEEP // 16, 16).T
    shared['idx16'] = np.ascontiguousarray(np.tile(wrapped, (8, 1)).astype(np.int16))
    shared['Wqkv'] = _bf(np.concatenate([inp['w_cq'], inp['w_ckv']], axis=1))
    shared['bgate'] = np.asarray(inp['b_gate'], np.float32)[None, :]

    qs = np.asarray(inp['q_scale'], np.float32)[:, None]
    kvs = np.asarray(inp['kv_scale'], np.float32)[:, None]
    maps = []
    for c in range(N_CORES):
        hs = [2 * c, 2 * c + 1]
        m = dict(shared)
        m['Wdq'] = _bf(np.concatenate(
            [qs * np.concatenate(
                [inp['w_dq_nope'][:, h * NOPE:(h + 1) * NOPE],
                 inp['w_dq_rope'][:, h * ROPE_D:(h + 1) * ROPE_D]], axis=1)
             for h in hs], axis=1))
        m['Wdk'] = _bf(np.concatenate(
            [kvs * np.concatenate(
                [inp['w_dk_nope'][:, h * NOPE:(h + 1) * NOPE],
                 inp['w_k_rope'][:, h * ROPE_D:(h + 1) * ROPE_D]], axis=1)
             for h in hs], axis=1))
        m['Wdv'] = _bf(np.concatenate(
            [kvs * inp['w_dv'][:, h * VHD:(h + 1) * VHD] for h in hs], axis=1))
        m['Wwk'] = _bf(np.concatenate(
            [inp['w_win_k'][:, h * HEAD_D:(h + 1) * HEAD_D] for h in hs], axis=1))
        m['Wsk'] = _bf(np.concatenate(
            [inp['w_sel_k'][:, h * HEAD_D:(h + 1) * HEAD_D] for h in hs], axis=1))
        m['Wwv_gi'] = _bf(np.concatenate(
            [inp['w_win_v'][:, h * VHD:(h + 1) * VHD] for h in hs]
            + [inp['w_gate'], inp['w_imp']], axis=1))
        m['Wsv'] = _bf(np.concatenate(
            [inp['w_sel_v'][:, h * VHD:(h + 1) * VHD] for h in hs], axis=1))
        m['Wproj'] = _bf(np.concatenate(
            [inp['w_proj'][h * VHD:(h + 1) * VHD, :] for h in hs], axis=1))
        maps.append(m)
    return maps


def run(inputs, **kw):
    nc = _build()
    in_maps = host_prep(inputs)
    res = bass_utils.run_bass_kernel_spmd(nc, in_maps, core_ids=list(range(N_CORES)),
                                          **kw)
    y = np.zeros((T, C), np.float32)
    for r in res.results:
        y += r['y']
    return y[None].astype(np.float32), res


def kernel(**inputs):
    y, _ = run(inputs)
    return y


# revision 55
# speedup vs baseline: 1.0310x; 1.0310x over previous
"""Trainium2 Bass kernel for nn_Attn_33028298506245 (sparse MLA-style attention).

Sharding: tensor-parallel over the 16 heads -> 2 heads per NeuronCore (8 cores).
Shared work (compressed q/kv projection, rmsnorm stats, gate, token top-k
selection gather) is replicated per core; the final out-projection is computed
per-core on that core's head slice and the partial [T, C] outputs are summed on
the host (the all-reduce / unshard step).

Device algorithm (per core, transposed "d-on-partitions" orientation):
  - qkv_nT = [w_cq|w_ckv]^T x^T               [128, T]  (rows 0:96 q, 96:128 kv)
  - rms stats via ones-mask matmul; inv scales folded into Q^T/K^T columns
  - Q^T/K^T produced directly transposed; RoPE applied in [d, T] layout
  - attention computed as S^T = K^T_block^T Q^T  [keys, queries]; softmax along
    the key (partition) axis via exp + ones-row-augmented V matmul (denominator
    comes out as row 96 of the PV accumulation); causal masking via a
    precomputed staircase strip multiply on the 4 diagonal key-blocks
  - branch 2 uses dma_gather (transpose=True) to build sel^T from the top-512
    tokens; rope positions are 0..511 as in the reference
  - gated combine of the 3 branches, then out-proj on the head slice
"""
import os
import sys

for _p in ("/opt/trn_rl_repo", "/root/.axon_site/_ro/trn_rl_repo"):
    if os.path.isdir(_p) and _p not in sys.path:
        sys.path.append(_p)

import numpy as np
import ml_dtypes

import concourse.bass as bass
import concourse.mybir as mybir
import concourse.tile as tile
from concourse import bacc
from concourse import bass_utils
from concourse.masks import make_identity

bf16 = ml_dtypes.bfloat16
FP32 = mybir.dt.float32
BF16 = mybir.dt.bfloat16

B, T, C = 1, 2048, 1024
H = 16
NOPE, ROPE_D, VHD = 32, 64, 96
KV_RANK, Q_RANK = 32, 96
S_KEEP = 512
EPS = 1e-6
HEAD_D = NOPE + ROPE_D          # 96
SCALE = 1.0 / float(np.sqrt(HEAD_D))
N_CORES = 8
NS = 4                          # strips of 512 queries
SW = 512                        # strip width
TB = 16                         # 128-token blocks
CB = 8                          # 128-channel blocks
VA = VHD + 1                    # V augmented with ones row -> denominator

AF = mybir.ActivationFunctionType
OP = mybir.AluOpType


def _emit(nc):
    dt_in = {}

    def din(name, shape, dtype):
        t = nc.dram_tensor(name, shape, dtype, kind="ExternalInput")
        dt_in[name] = t
        return t

    xT_d = din("xT", [C, T], BF16)
    xbf_d = din("x_bf", [T, C], BF16)
    cos_d = din("cos_t", [32, T], BF16)
    sin_d = din("sin_t", [32, T], BF16)
    mask_d = din("mask_strip", [128, 128], BF16)
    idx_d = din("idx16", [128, S_KEEP // 16], mybir.dt.int16)
    wqkv_d = din("Wqkv", [C, 128], BF16)
    wdq_d = din("Wdq", [Q_RANK, 192], BF16)
    wdk_d = din("Wdk", [KV_RANK, 192], BF16)
    wdv_d = din("Wdv", [KV_RANK, 192], BF16)
    wwk_d = din("Wwk", [C, 192], BF16)
    wsk_d = din("Wsk", [C, 192], BF16)
    wwvgi_d = din("Wwv_gi", [C, 196], BF16)
    wsv_d = din("Wsv", [C, 192], BF16)
    wproj_d = din("Wproj", [Q_RANK, 2 * C], BF16)
    bgate_d = din("bgate", [1, 3], FP32)
    y_d = nc.dram_tensor("y", [T, C], FP32, kind="ExternalOutput")

    with tile.TileContext(nc) as tc:
        _body(nc, tc, xT_d, xbf_d, cos_d, sin_d, mask_d, idx_d, wqkv_d, wdq_d,
              wdk_d, wdv_d, wwk_d, wsk_d, wwvgi_d, wsv_d, wproj_d, bgate_d, y_d)
    return dt_in


def _body(nc, tc, xT_d, xbf_d, cos_d, sin_d, mask_d, idx_d, wqkv_d, wdq_d,
          wdk_d, wdv_d, wwk_d, wsk_d, wwvgi_d, wsv_d, wproj_d, bgate_d, y_d):
    from contextlib import ExitStack
    ctx = ExitStack()
    with ctx:
        const = ctx.enter_context(tc.tile_pool(name="const", bufs=1))
        sbA = ctx.enter_context(tc.tile_pool(name="sbA", bufs=1))
        work = ctx.enter_context(tc.tile_pool(name="work", bufs=2))
        ropep = ctx.enter_context(tc.tile_pool(name="ropep", bufs=3))
        expp = ctx.enter_context(tc.tile_pool(name="expp", bufs=4))
        bcp = ctx.enter_context(tc.tile_pool(name="bcp", bufs=2))
        recp = ctx.enter_context(tc.tile_pool(name="recp", bufs=4))
        otp = ctx.enter_context(tc.tile_pool(name="otp", bufs=2))
        ysb = ctx.enter_context(tc.tile_pool(name="ysb", bufs=2))
        psS = ctx.enter_context(tc.tile_pool(name="psS", bufs=2, space="PSUM"))
        psO = ctx.enter_context(tc.tile_pool(name="psO", bufs=2, space="PSUM"))
        psP = ctx.enter_context(tc.tile_pool(name="psP", bufs=2, space="PSUM"))

        # ---- constants / weights into SBUF ----
        # weights needed first come first; x^T is loaded strip-major in the
        # order the A-phase consumes it so the first matmul starts early
        wqkv_sb = const.tile([128, CB, 128], BF16)
        nc.sync.dma_start(wqkv_sb, wqkv_d.ap().rearrange("(cb p) m -> p cb m", p=128))
        xT_sb = const.tile([128, CB, T], BF16)
        for st in range(NS):
            for cb in range(CB):
                nc.sync.dma_start(xT_sb[:, cb, st * SW:(st + 1) * SW],
                                  xT_d.ap()[cb * 128:(cb + 1) * 128,
                                            st * SW:(st + 1) * SW])
        cos_sb = const.tile([32, T], FP32)
        sin_sb = const.tile([32, T], FP32)
        nc.sync.dma_start(cos_sb, cos_d.ap())
        nc.sync.dma_start(sin_sb, sin_d.ap())
        mask_sb = const.tile([128, 128], BF16)
        nc.sync.dma_start(mask_sb, mask_d.ap())
        idx_sb = const.tile([128, S_KEEP // 16], mybir.dt.int16)
        nc.sync.dma_start(idx_sb, idx_d.ap())

        wdq_sb = const.tile([Q_RANK, 192], BF16)
        nc.sync.dma_start(wdq_sb, wdq_d.ap())
        wdk_sb = const.tile([KV_RANK, 192], BF16)
        nc.sync.dma_start(wdk_sb, wdk_d.ap())
        wdv_sb = const.tile([KV_RANK, 192], BF16)
        nc.sync.dma_start(wdv_sb, wdv_d.ap())
        wwk_sb = const.tile([128, CB, 192], BF16)
        nc.sync.dma_start(wwk_sb, wwk_d.ap().rearrange("(cb p) m -> p cb m", p=128))
        wsk_sb = const.tile([128, CB, 192], BF16)
        nc.sync.dma_start(wsk_sb, wsk_d.ap().rearrange("(cb p) m -> p cb m", p=128))
        wwvgi_sb = const.tile([128, CB, 196], BF16)
        nc.sync.dma_start(wwvgi_sb, wwvgi_d.ap().rearrange("(cb p) m -> p cb m", p=128))
        wsv_sb = const.tile([128, CB, 192], BF16)
        nc.sync.dma_start(wsv_sb, wsv_d.ap().rearrange("(cb p) m -> p cb m", p=128))
        wproj_sb = const.tile([Q_RANK, 2 * C], BF16)
        nc.sync.dma_start(wproj_sb, wproj_d.ap())
        bgate_sb = const.tile([1, 3], FP32)
        nc.sync.dma_start(bgate_sb, bgate_d.ap())

        ones96_bf = const.tile([Q_RANK, 1], BF16)
        nc.vector.memset(ones96_bf, 1.0)
        ones32_bf = const.tile([KV_RANK, 1], BF16)
        nc.vector.memset(ones32_bf, 1.0)
        ones128_f32 = const.tile([128, 1], FP32)
        nc.vector.memset(ones128_f32, 1.0)
        ident1 = const.tile([1, 1], FP32)
        nc.vector.memset(ident1, 1.0)

        # ---- persistent intermediates ----
        qnT = sbA.tile([Q_RANK, T], BF16)               # q_nT (unnormalized)
        kvT = sbA.tile([KV_RANK, T], BF16)              # kv_nT (unnormalized)
        selT = sbA.tile([128, CB, S_KEEP], BF16)        # sel^T gathered
        inv_q_rows = [sbA.tile([1, SW], FP32, tag=f"invq{s}", name=f"invq{s}") for s in range(NS)]
        inv_kv_rows = [sbA.tile([1, SW], FP32, tag=f"invk{s}", name=f"invk{s}") for s in range(NS)]
        inv_colT = sbA.tile([128, TB], FP32)            # per-token kv inv (columns)
        gi_acc = sbA.tile([128, 4], FP32)
        gate_sb = sbA.tile([1, 3], FP32)
        QT = [sbA.tile([Q_RANK, T], BF16, tag=f"QT{h}", name=f"QT{h}") for h in range(2)]
        K1T = [sbA.tile([Q_RANK, T], BF16, tag=f"K1T{h}", name=f"K1T{h}") for h in range(2)]
        KwT = [sbA.tile([Q_RANK, T], BF16, tag=f"KwT{h}", name=f"KwT{h}") for h in range(2)]
        KsT = [sbA.tile([Q_RANK, S_KEEP], BF16, tag=f"KsT{h}", name=f"KsT{h}") for h in range(2)]
        V1 = [sbA.tile([128, TB, VA], BF16, tag=f"V1{h}", name=f"V1_{h}") for h in range(2)]
        Vw = [sbA.tile([128, TB, VA], BF16, tag=f"Vw{h}", name=f"Vw_{h}") for h in range(2)]
        Vs = [sbA.tile([128, 4, VA], BF16, tag=f"Vs{h}", name=f"Vs_{h}") for h in range(2)]
        OTt = {(h, s): sbA.tile([Q_RANK, SW], BF16, tag=f"OTt{h}_{s}", name=f"OTt{h}_{s}")
               for h in range(2) for s in range(NS)}

        # ones rows of augmented V (never overwritten afterwards)
        for h in range(2):
            nc.vector.memset(V1[h][:, :, VHD:VA], 1.0)
            nc.vector.memset(Vw[h][:, :, VHD:VA], 1.0)
            nc.vector.memset(Vs[h][:, :, VHD:VA], 1.0)
        nc.vector.memset(gi_acc, 0.0)

        # ---- A: shared projection + rms stats per strip ----
        for s in range(NS):
            t0 = s * SW
            ps = psP.tile([128, SW], FP32, tag="pp")
            for cb in range(CB):
                nc.tensor.matmul(ps, wqkv_sb[:, cb, :], xT_sb[:, cb, t0:t0 + SW],
                                 start=(cb == 0), stop=(cb == CB - 1))
            nc.scalar.copy(qnT[:, t0:t0 + SW], ps[0:Q_RANK, :])
            # 32-partition quadrant move (q3 -> q0) on DVE
            nc.vector.tensor_copy(kvT[:, t0:t0 + SW], ps[Q_RANK:128, :])
            qsq = work.tile([Q_RANK, SW], BF16, tag="qsq")
            nc.vector.tensor_mul(qsq, qnT[:, t0:t0 + SW], qnT[:, t0:t0 + SW])
            ksq = work.tile([KV_RANK, SW], BF16, tag="ksq")
            nc.vector.tensor_mul(ksq, kvT[:, t0:t0 + SW], kvT[:, t0:t0 + SW])
            for (onev, sqt, invr, rk) in ((ones96_bf, qsq, inv_q_rows[s], Q_RANK),
                                          (ones32_bf, ksq, inv_kv_rows[s], KV_RANK)):
                ssq = psO.tile([1, SW], FP32, tag="ot")
                nc.tensor.matmul(ssq, onev, sqt, start=True, stop=True)
                mtmp = work.tile([1, SW], FP32, tag="mtmp")
                nc.vector.tensor_scalar(mtmp, ssq, 1.0 / rk, EPS,
                                        op0=OP.mult, op1=OP.add)
                stmp = work.tile([1, SW], FP32, tag="stmp")
                nc.scalar.activation(stmp, mtmp, AF.Sqrt)
                nc.vector.reciprocal(invr, stmp)
            # transpose kv inv to column form for V1 scaling
            for b in range(4):
                tp = psP.tile([128, 1], FP32, tag="pp")
                nc.tensor.transpose(tp, inv_kv_rows[s][:, b * 128:(b + 1) * 128],
                                    ident1)
                nc.scalar.copy(inv_colT[:, s * 4 + b:s * 4 + b + 1], tp)

        # ---- Vwin + gate/imp projection (heads paired, 2 blocks/psum) ----
        for b2 in range(TB // 2):
            ps = psP.tile([128, 2, 196], FP32, tag="pp", name=f"pvw{b2}")
            for j in range(2):
                b = b2 * 2 + j
                for cb in range(CB):
                    nc.tensor.matmul(ps[:, j, :],
                                     xT_sb[:, cb, b * 128:(b + 1) * 128],
                                     wwvgi_sb[:, cb, :],
                                     start=(cb == 0), stop=(cb == CB - 1))
            for j in range(2):
                b = b2 * 2 + j
                for h in range(2):
                    nc.scalar.copy(Vw[h][:, b, 0:VHD],
                                   ps[:, j, h * 96:h * 96 + 96])
                nc.vector.tensor_add(gi_acc, gi_acc, ps[:, j, 192:196])

        # gate = softmax(sum/T + b_gate)
        glp = psO.tile([1, 4], FP32, tag="ot")
        nc.tensor.matmul(glp, ones128_f32, gi_acc, start=True, stop=True)
        gl = work.tile([1, 4], FP32, tag="gl")
        nc.scalar.activation(gl, glp, AF.Copy, scale=1.0 / T)
        nc.vector.tensor_add(gl[0:1, 0:3], gl[0:1, 0:3], bgate_sb)
        ge = work.tile([1, 3], FP32, tag="ge")
        nc.scalar.activation(ge, gl[0:1, 0:3], AF.Exp)
        gs = work.tile([1, 1], FP32, tag="gs")
        nc.vector.reduce_sum(gs, ge, axis=mybir.AxisListType.X)
        gr = work.tile([1, 1], FP32, tag="gr")
        nc.vector.reciprocal(gr, gs)
        nc.vector.tensor_scalar_mul(gate_sb, ge, gr)

        # ---- top-k gather: selT[p, cb, i] = x_bf[idx[i], cb*128+p] ----
        nc.gpsimd.dma_gather(
            out_ap=selT[:],
            in_ap=xbf_d.ap(),
            idxs_ap=idx_sb[:],
            num_idxs=S_KEEP,
            num_idxs_reg=S_KEEP,
            elem_size=C,
            transpose=True,
        )

        # ---- Vsel (heads paired, 2 blocks/psum) ----
        for b2 in range(2):
            ps = psP.tile([128, 2, 192], FP32, tag="pp", name=f"pvs{b2}")
            for j in range(2):
                b = b2 * 2 + j
                for cb in range(CB):
                    nc.tensor.matmul(ps[:, j, :],
                                     selT[:, cb, b * 128:(b + 1) * 128],
                                     wsv_sb[:, cb, :],
                                     start=(cb == 0), stop=(cb == CB - 1))
            for j in range(2):
                b = b2 * 2 + j
                for h in range(2):
                    nc.scalar.copy(Vs[h][:, b, 0:VHD],
                                   ps[:, j, h * 96:h * 96 + 96])

        # ---- V1 (heads paired, 2 blocks/psum, kv inv scaling at evict) ----
        for b2 in range(TB // 2):
            ps = psP.tile([128, 2, 192], FP32, tag="pp", name=f"pv1{b2}")
            for j in range(2):
                b = b2 * 2 + j
                nc.tensor.matmul(ps[:, j, :], kvT[:, b * 128:(b + 1) * 128],
                                 wdv_sb, start=True, stop=True)
            for j in range(2):
                b = b2 * 2 + j
                for h in range(2):
                    nc.vector.tensor_scalar_mul(V1[h][:, b, 0:VHD],
                                                ps[:, j, h * 96:h * 96 + 96],
                                                inv_colT[:, b:b + 1])

        def rope_from(ps, out_t, t0, cos_slc, sin_slc, inv_bc):
            """Evict [96, w] psum -> out_t cols t0:t0+w applying optional
            per-column inv scaling (inv_bc broadcast tile or None) + RoPE on
            rows 32:96."""
            w = cos_slc.shape[-1]
            if inv_bc is not None:
                nc.vector.tensor_mul(out_t[0:32, t0:t0 + w], ps[0:32, :],
                                     inv_bc[0:32, :])
                p1 = ropep.tile([32, SW], FP32, tag="p1")
                p2 = ropep.tile([32, SW], FP32, tag="p2")
                nc.vector.tensor_mul(p1[:, 0:w], ps[32:64, :], inv_bc[32:64, :])
                nc.vector.tensor_mul(p2[:, 0:w], ps[64:96, :], inv_bc[64:96, :])
                r, i = p1[:, 0:w], p2[:, 0:w]
            else:
                nc.scalar.copy(out_t[0:32, t0:t0 + w], ps[0:32, :])
                r, i = ps[32:64, :], ps[64:96, :]
            ta = ropep.tile([32, SW], FP32, tag="ta")
            tb = ropep.tile([32, SW], FP32, tag="tb")
            nc.vector.tensor_mul(ta[:, 0:w], r, cos_slc)
            nc.vector.tensor_mul(tb[:, 0:w], i, sin_slc)
            nc.vector.tensor_sub(out_t[32:64, t0:t0 + w], ta[:, 0:w], tb[:, 0:w])
            nc.vector.tensor_mul(ta[:, 0:w], r, sin_slc)
            nc.vector.tensor_mul(tb[:, 0:w], i, cos_slc)
            nc.vector.tensor_add(out_t[64:96, t0:t0 + w], ta[:, 0:w], tb[:, 0:w])

        # ---- per-head projections (both heads) ----
        def d_proj(h):
            sl = slice(h * 96, h * 96 + 96)
            for s2 in range(NS // 2):
                t0 = s2 * 2 * SW
                W2 = 2 * SW
                for (wt, src, out_t, ib) in (
                        (wdq_sb[:, sl], qnT, QT[h], ibq_f),
                        (wdk_sb[:, sl], kvT, K1T[h], ibk_f)):
                    stg = ropep.tile([Q_RANK, 2 * SW], BF16, tag="stg",
                                     name=f"stgd{h}{s2}")
                    for j in range(2):
                        ts_ = t0 + j * SW
                        ps = psP.tile([Q_RANK, SW], FP32, tag="pp",
                                      name=f"psd{h}{s2}{j}")
                        nc.tensor.matmul(ps, wt, src[:, ts_:ts_ + SW],
                                         start=True, stop=True)
                        nc.scalar.copy(stg[:, j * SW:(j + 1) * SW], ps)
                    rope_stg(stg, out_t, t0, W2, t0,
                             ib[:, t0:t0 + W2] if ib is not None else None)
                stg = ropep.tile([Q_RANK, 2 * SW], BF16, tag="stg", name=f"stgw{h}{s2}")
                for j in range(2):
                    ts_ = t0 + j * SW
                    ps = psP.tile([Q_RANK, SW], FP32, tag="pp", name=f"psw{h}{s2}{j}")
                    for cb in range(CB):
                        nc.tensor.matmul(ps, wwk_sb[:, cb, sl],
                                         xT_sb[:, cb, ts_:ts_ + SW],
                                         start=(cb == 0), stop=(cb == CB - 1))
                    nc.scalar.copy(stg[:, j * SW:(j + 1) * SW], ps)
                rope_stg(stg, KwT[h], t0, W2, t0, None)

            # Ksel^T (rope positions 0..S_KEEP-1)
            stg = ropep.tile([Q_RANK, 2 * SW], BF16, tag="stg", name=f"stgs{h}")
            ps = psP.tile([Q_RANK, S_KEEP], FP32, tag="pp", name=f"pss{h}")
            for cb in range(CB):
                nc.tensor.matmul(ps, wsk_sb[:, cb, sl], selT[:, cb, :],
                                 start=(cb == 0), stop=(cb == CB - 1))
            nc.scalar.copy(stg[:, 0:S_KEEP], ps)
            rope_stg(stg, KsT[h], 0, S_KEEP, 0, None)

        # ---- attention per head (projections emitted just before each) ----
        for h in range(2):
            d_proj(h)
            # attention: 3 branches per strip
            for s in range(NS):
                t0 = s * SW
                qs = QT[h][:, t0:t0 + SW]
                otacc = None
                for br, (KT, Vt, causal) in enumerate([
                        (K1T[h], V1[h], True),
                        (KsT[h], Vs[h], False),
                        (KwT[h], Vw[h], True)]):
                    nk = 4 * (s + 1) if causal else 4
                    otp_ps = psO.tile([VA, SW], FP32, tag="ot")
                    for pair in range(nk // 2):
                        # diagonal key-blocks only need columns >= c0 (keys can
                        # never attend to earlier queries); the rest is masked
                        def col0(kb):
                            return (max(0, kb * 128 - s * SW)
                                    if causal and kb >= nk - 4 else 0)
                        kbs = (pair * 2, pair * 2 + 1)
                        c0s = [col0(kb) for kb in kbs]
                        cu = min(c0s)
                        sp = psS.tile([128, 2, SW], FP32, tag="sp")
                        for j, kb in enumerate(kbs):
                            nc.tensor.matmul(sp[:, j, c0s[j]:SW],
                                             KT[:, kb * 128:(kb + 1) * 128],
                                             qs[:, c0s[j]:SW],
                                             start=True, stop=True)
                        es = expp.tile([128, 2, SW], BF16, tag="es")
                        if c0s[0] == c0s[1]:
                            nc.scalar.activation(es[:, :, cu:SW], sp[:, :, cu:SW],
                                                 AF.Exp, scale=SCALE)
                        else:
                            for j in range(2):
                                nc.scalar.activation(es[:, j, c0s[j]:SW],
                                                     sp[:, j, c0s[j]:SW],
                                                     AF.Exp, scale=SCALE)
                        for j, kb in enumerate(kbs):
                            c0 = c0s[j]
                            if causal and kb >= nk - 4:
                                # partial triangle lives in the first 128 cols
                                nc.vector.tensor_mul(es[:, j, c0:c0 + 128],
                                                     es[:, j, c0:c0 + 128],
                                                     mask_sb[:, 0:128])
                            nc.tensor.matmul(otp_ps[:, c0:SW], Vt[:, kb, :],
                                             es[:, j, c0:SW],
                                             start=(kb == 0), stop=(kb == nk - 1))
                    rec = recp.tile([1, SW], FP32, tag="rec")
                    nc.vector.reciprocal(rec, otp_ps[VHD:VA, :])
                    rsc = recp.tile([1, SW], FP32, tag="rsc")
                    nc.vector.tensor_scalar_mul(rsc, rec, gate_sb[0:1, br:br + 1])
                    obc = bcp.tile([96, SW], FP32, tag="obc")
                    nc.gpsimd.partition_broadcast(obc, rsc)
                    if br == 0:
                        otacc = otp.tile([96, SW], FP32, tag="otacc")
                        nc.vector.tensor_mul(otacc, otp_ps[0:VHD, :], obc)
                    else:
                        tmp = otp.tile([96, SW], FP32, tag="otmp")
                        nc.vector.tensor_mul(tmp, otp_ps[0:VHD, :], obc)
                        dst = OTt[(h, s)] if br == 2 else otacc
                        nc.vector.tensor_add(dst, otacc, tmp)

        # ---- out-projection (partial y on the 2-head slice) ----
        for b in range(TB):
            s = b // 4
            c0 = (b % 4) * 128
            yt = ysb.tile([128, C], FP32, tag="yt")
            for half in range(2):
                yp = psP.tile([128, SW], FP32, tag="pp")
                for h in range(2):
                    nc.tensor.matmul(yp, OTt[(h, s)][:, c0:c0 + 128],
                                     wproj_sb[:, h * C + half * SW:
                                              h * C + half * SW + SW],
                                     start=(h == 0), stop=(h == 1))
                nc.vector.tensor_copy(yt[:, half * SW:half * SW + SW], yp)
            nc.sync.dma_start(y_d.ap()[b * 128:(b + 1) * 128, :], yt)


# ------------------------------------------------------------------
# host side
# ------------------------------------------------------------------

_BUILT = None


def _build():
    global _BUILT
    if _BUILT is None:
        nc = bacc.Bacc(None, target_bir_lowering=False, debug=False)
        _emit(nc)
        nc.finalize()
        _BUILT = nc
    return _BUILT


def _bf(x):
    return np.ascontiguousarray(np.asarray(x, np.float32)).astype(bf16)


def host_prep(inputs):
    inp = {k: np.asarray(v, np.float32) if np.asarray(v).dtype == np.float32
           else np.asarray(v) for k, v in inputs.items()}
    x = np.asarray(inp['x'], np.float32)[0]          # [T, C]
    shared = {}
    shared['xT'] = _bf(x.T)
    shared['x_bf'] = _bf(x)
    inv = 1.0 / (10000.0 ** (np.arange(0, ROPE_D, 2, dtype=np.float32) / ROPE_D))
    f = np.outer(inv, np.arange(T, dtype=np.float32))
    shared['cos_t'] = np.cos(f).astype(bf16)
    shared['sin_t'] = np.sin(f).astype(bf16)
    k = np.arange(128)[:, None]
    cgrid = np.arange(128)[None, :]
    shared['mask_strip'] = (k <= cgrid).astype(bf16)
    scores = x @ np.asarray(inp['w_imp'], np.float32)[:, 0]
    idx = np.sort(np.argsort(-scores)[:S_KEEP])
    wrapped = idx.astype(np.int16).reshape(S_KEEP // 16, 16).T
    shared['idx16'] = np.ascontiguousarray(np.tile(wrapped, (8, 1)).astype(np.int16))
    shared['Wqkv'] = _bf(np.concatenate([inp['w_cq'], inp['w_ckv']], axis=1))
    shared['bgate'] = np.asarray(inp['b_gate'], np.float32)[None, :]

    qs = np.asarray(inp['q_scale'], np.float32)[:, None]
    kvs = np.asarray(inp['kv_scale'], np.float32)[:, None]
    maps = []
    for c in range(N_CORES):
        hs = [2 * c, 2 * c + 1]
        m = dict(shared)
        m['Wdq'] = _bf(np.concatenate(
            [qs * np.concatenate(
                [inp['w_dq_nope'][:, h * NOPE:(h + 1) * NOPE],
                 inp['w_dq_rope'][:, h * ROPE_D:(h + 1) * ROPE_D]], axis=1)
             for h in hs], axis=1))
        m['Wdk'] = _bf(np.concatenate(
            [kvs * np.concatenate(
                [inp['w_dk_nope'][:, h * NOPE:(h + 1) * NOPE],
                 inp['w_k_rope'][:, h * ROPE_D:(h + 1) * ROPE_D]], axis=1)
             for h in hs], axis=1))
        m['Wdv'] = _bf(np.concatenate(
            [kvs * inp['w_dv'][:, h * VHD:(h + 1) * VHD] for h in hs], axis=1))
        m['Wwk'] = _bf(np.concatenate(
            [inp['w_win_k'][:, h * HEAD_D:(h + 1) * HEAD_D] for h in hs], axis=1))
        m['Wsk'] = _bf(np.concatenate(
            [inp['w_sel_k'][:, h * HEAD_D:(h + 1) * HEAD_D] for h in hs], axis=1))
        m['Wwv_gi'] = _bf(np.concatenate(
            [inp['w_win_v'][:, h * VHD:(h + 1) * VHD] for h in hs]
            + [inp['w_gate'], inp['w_imp']], axis=1))
        m['Wsv'] = _bf(np.concatenate(
            [inp['w_sel_v'][:, h * VHD:(h + 1) * VHD] for h in hs], axis=1))
        m['Wproj'] = _bf(np.concatenate(
            [inp['w_proj'][h * VHD:(h + 1) * VHD, :] for h in hs], axis=1))
        maps.append(m)
    return maps


def run(inputs, **kw):
    nc = _build()
    in_maps = host_prep(inputs)
    res = bass_utils.run_bass_kernel_spmd(nc, in_maps, core_ids=list(range(N_CORES)),
                                          **kw)
    y = np.zeros((T, C), np.float32)
    for r in res.results:
        y += r['y']
    return y[None].astype(np.float32), res


def kernel(**inputs):
    y, _ = run(inputs)
    return y


# revision 66
# speedup vs baseline: 1.2113x; 1.1749x over previous
"""Trainium2 Bass kernel for nn_Attn_33028298506245 (sparse MLA-style attention).

Sharding: tensor-parallel over the 16 heads -> 2 heads per NeuronCore (8 cores).
Shared work (compressed q/kv projection, rmsnorm stats, gate, token top-k
selection gather) is replicated per core; the final out-projection is computed
per-core on that core's head slice and the partial [T, C] outputs are summed on
the host (the all-reduce / unshard step).

Device algorithm (per core, transposed "d-on-partitions" orientation):
  - qkv_nT = [w_cq|w_ckv]^T x^T               [128, T]  (rows 0:96 q, 96:128 kv)
  - rms stats via ones-mask matmul; inv scales folded into Q^T/K^T columns
  - Q^T/K^T produced directly transposed; RoPE applied in [d, T] layout
  - attention computed as S^T = K^T_block^T Q^T  [keys, queries]; softmax along
    the key (partition) axis via exp + ones-row-augmented V matmul (denominator
    comes out as row 96 of the PV accumulation); causal masking via a
    precomputed staircase strip multiply on the 4 diagonal key-blocks
  - branch 2 uses dma_gather (transpose=True) to build sel^T from the top-512
    tokens; rope positions are 0..511 as in the reference
  - gated combine of the 3 branches, then out-proj on the head slice
"""
import os
import sys

for _p in ("/opt/trn_rl_repo", "/root/.axon_site/_ro/trn_rl_repo"):
    if os.path.isdir(_p) and _p not in sys.path:
        sys.path.append(_p)

import numpy as np
import ml_dtypes

import concourse.mybir as mybir
import concourse.tile as tile
from concourse import bacc
from concourse import bass_utils

bf16 = ml_dtypes.bfloat16
FP32 = mybir.dt.float32
BF16 = mybir.dt.bfloat16

B, T, C = 1, 2048, 1024
H = 16
NOPE, ROPE_D, VHD = 32, 64, 96
KV_RANK, Q_RANK = 32, 96
S_KEEP = 512
EPS = 1e-6
HEAD_D = NOPE + ROPE_D          # 96
SCALE = 1.0 / float(np.sqrt(HEAD_D))
N_CORES = 8
NS = 4                          # strips of 512 queries
SW = 512                        # strip width
TB = 16                         # 128-token blocks
CB = 8                          # 128-channel blocks
VA = VHD + 1                    # V augmented with ones row -> denominator

AF = mybir.ActivationFunctionType
OP = mybir.AluOpType


def _emit(nc):
    dt_in = {}

    def din(name, shape, dtype):
        t = nc.dram_tensor(name, shape, dtype, kind="ExternalInput")
        dt_in[name] = t
        return t

    xT_d = din("xT", [C, T], BF16)
    xbf_d = din("x_bf", [T, C], BF16)
    cos_d = din("cos_t", [32, T], BF16)
    sin_d = din("sin_t", [32, T], BF16)
    mask_d = din("mask_strip", [128, 128], BF16)
    idx_d = din("idx16", [128, S_KEEP // 16], mybir.dt.int16)
    wqkv_d = din("Wqkv", [C, 128], BF16)
    wdq_d = din("Wdq", [Q_RANK, 192], BF16)
    wdk_d = din("Wdk", [KV_RANK, 192], BF16)
    wdv_d = din("Wdv", [KV_RANK, 192], BF16)
    wwk_d = din("Wwk", [C, 192], BF16)
    wsk_d = din("Wsk", [C, 192], BF16)
    wwvgi_d = din("Wwv_gi", [C, 196], BF16)
    wsv_d = din("Wsv", [C, 192], BF16)
    wproj_d = din("Wproj", [Q_RANK, 2 * C], BF16)
    bgate_d = din("bgate", [1, 3], FP32)
    y_d = nc.dram_tensor("y", [T, C], FP32, kind="ExternalOutput")

    with tile.TileContext(nc) as tc:
        _body(nc, tc, xT_d, xbf_d, cos_d, sin_d, mask_d, idx_d, wqkv_d, wdq_d,
              wdk_d, wdv_d, wwk_d, wsk_d, wwvgi_d, wsv_d, wproj_d, bgate_d, y_d)
    return dt_in


def _body(nc, tc, xT_d, xbf_d, cos_d, sin_d, mask_d, idx_d, wqkv_d, wdq_d,
          wdk_d, wdv_d, wwk_d, wsk_d, wwvgi_d, wsv_d, wproj_d, bgate_d, y_d):
    from contextlib import ExitStack
    ctx = ExitStack()
    with ctx:
        const = ctx.enter_context(tc.tile_pool(name="const", bufs=1))
        sbA = ctx.enter_context(tc.tile_pool(name="sbA", bufs=1))
        work = ctx.enter_context(tc.tile_pool(name="work", bufs=2))
        ropep = ctx.enter_context(tc.tile_pool(name="ropep", bufs=3))
        expp = ctx.enter_context(tc.tile_pool(name="expp", bufs=5))
        bcp = ctx.enter_context(tc.tile_pool(name="bcp", bufs=2))
        recp = ctx.enter_context(tc.tile_pool(name="recp", bufs=4))
        otp = ctx.enter_context(tc.tile_pool(name="otp", bufs=2))
        ysb = ctx.enter_context(tc.tile_pool(name="ysb", bufs=3))
        psS = ctx.enter_context(tc.tile_pool(name="psS", bufs=2, space="PSUM"))
        psO = ctx.enter_context(tc.tile_pool(name="psO", bufs=2, space="PSUM"))
        psP = ctx.enter_context(tc.tile_pool(name="psP", bufs=2, space="PSUM"))

        # ---- constants / weights into SBUF ----
        # weights needed first come first; x^T is loaded strip-major in the
        # order the A-phase consumes it so the first matmul starts early
        wqkv_sb = const.tile([128, CB, 128], BF16)
        nc.sync.dma_start(wqkv_sb, wqkv_d.ap().rearrange("(cb p) m -> p cb m", p=128))
        xT_sb = const.tile([128, CB, T], BF16)
        for st in range(NS):
            for cb in range(CB):
                nc.sync.dma_start(xT_sb[:, cb, st * SW:(st + 1) * SW],
                                  xT_d.ap()[cb * 128:(cb + 1) * 128,
                                            st * SW:(st + 1) * SW])
        cos_sb = const.tile([32, T], FP32)
        sin_sb = const.tile([32, T], FP32)
        nc.sync.dma_start(cos_sb, cos_d.ap())
        nc.sync.dma_start(sin_sb, sin_d.ap())
        mask_sb = const.tile([128, 128], BF16)
        nc.sync.dma_start(mask_sb, mask_d.ap())
        idx_sb = const.tile([128, S_KEEP // 16], mybir.dt.int16)
        nc.sync.dma_start(idx_sb, idx_d.ap())

        wdq_sb = const.tile([Q_RANK, 192], BF16)
        nc.sync.dma_start(wdq_sb, wdq_d.ap())
        wdk_sb = const.tile([KV_RANK, 192], BF16)
        nc.sync.dma_start(wdk_sb, wdk_d.ap())
        wdv_sb = const.tile([KV_RANK, 192], BF16)
        nc.sync.dma_start(wdv_sb, wdv_d.ap())
        wwk_sb = const.tile([128, CB, 192], BF16)
        nc.sync.dma_start(wwk_sb, wwk_d.ap().rearrange("(cb p) m -> p cb m", p=128))
        wsk_sb = const.tile([128, CB, 192], BF16)
        nc.sync.dma_start(wsk_sb, wsk_d.ap().rearrange("(cb p) m -> p cb m", p=128))
        wwvgi_sb = const.tile([128, CB, 196], BF16)
        nc.sync.dma_start(wwvgi_sb, wwvgi_d.ap().rearrange("(cb p) m -> p cb m", p=128))
        wsv_sb = const.tile([128, CB, 192], BF16)
        nc.sync.dma_start(wsv_sb, wsv_d.ap().rearrange("(cb p) m -> p cb m", p=128))
        wproj_sb = const.tile([Q_RANK, 2 * C], BF16)
        nc.sync.dma_start(wproj_sb, wproj_d.ap())
        bgate_sb = const.tile([1, 3], FP32)
        nc.sync.dma_start(bgate_sb, bgate_d.ap())

        ones96_bf = const.tile([Q_RANK, 1], BF16)
        nc.vector.memset(ones96_bf, 1.0)
        ones32_bf = const.tile([KV_RANK, 1], BF16)
        nc.vector.memset(ones32_bf, 1.0)
        ones128_f32 = const.tile([128, 1], FP32)
        nc.vector.memset(ones128_f32, 1.0)
        ident1 = const.tile([1, 1], FP32)
        nc.vector.memset(ident1, 1.0)

        # ---- persistent intermediates ----
        qnT = sbA.tile([Q_RANK, T], BF16)               # q_nT (unnormalized)
        kvT = sbA.tile([KV_RANK, T], BF16)              # kv_nT (unnormalized)
        selT = sbA.tile([128, CB, S_KEEP], BF16)        # sel^T gathered
        inv_q_rows = [sbA.tile([1, SW], FP32, tag=f"invq{s}", name=f"invq{s}") for s in range(NS)]
        inv_kv_rows = [sbA.tile([1, SW], FP32, tag=f"invk{s}", name=f"invk{s}") for s in range(NS)]
        inv_colT = sbA.tile([128, TB], FP32)            # per-token kv inv (columns)
        gi_acc = sbA.tile([128, 4], FP32)
        gate_sb = sbA.tile([1, 3], FP32)
        QT = [sbA.tile([Q_RANK, T], BF16, tag=f"QT{h}", name=f"QT{h}") for h in range(2)]
        K1T = [sbA.tile([Q_RANK, T], BF16, tag=f"K1T{h}", name=f"K1T{h}") for h in range(2)]
        KwT = [sbA.tile([Q_RANK, T], BF16, tag=f"KwT{h}", name=f"KwT{h}") for h in range(2)]
        KsT = [sbA.tile([Q_RANK, S_KEEP], BF16, tag=f"KsT{h}", name=f"KsT{h}") for h in range(2)]
        V1 = [sbA.tile([128, TB, VA], BF16, tag=f"V1{h}", name=f"V1_{h}") for h in range(2)]
        Vw = [sbA.tile([128, TB, VA], BF16, tag=f"Vw{h}", name=f"Vw_{h}") for h in range(2)]
        Vs = [sbA.tile([128, 4, VA], BF16, tag=f"Vs{h}", name=f"Vs_{h}") for h in range(2)]
        OTt = {(h, s): sbA.tile([Q_RANK, SW], BF16, tag=f"OTt{h}_{s}", name=f"OTt{h}_{s}")
               for h in range(2) for s in range(NS)}

        # ones rows of augmented V (never overwritten afterwards)
        for h in range(2):
            nc.vector.memset(V1[h][:, :, VHD:VA], 1.0)
            nc.vector.memset(Vw[h][:, :, VHD:VA], 1.0)
            nc.vector.memset(Vs[h][:, :, VHD:VA], 1.0)
        nc.vector.memset(gi_acc, 0.0)

        # ---- A: shared projection + rms stats per strip ----
        for s in range(NS):
            t0 = s * SW
            ps = psP.tile([128, SW], FP32, tag="pp")
            for cb in range(CB):
                nc.tensor.matmul(ps, wqkv_sb[:, cb, :], xT_sb[:, cb, t0:t0 + SW],
                                 start=(cb == 0), stop=(cb == CB - 1))
            nc.scalar.copy(qnT[:, t0:t0 + SW], ps[0:Q_RANK, :])
            # 32-partition quadrant move (q3 -> q0) on DVE
            nc.vector.tensor_copy(kvT[:, t0:t0 + SW], ps[Q_RANK:128, :])
            qsq = work.tile([Q_RANK, SW], BF16, tag="qsq")
            nc.vector.tensor_mul(qsq, qnT[:, t0:t0 + SW], qnT[:, t0:t0 + SW])
            ksq = work.tile([KV_RANK, SW], BF16, tag="ksq")
            nc.vector.tensor_mul(ksq, kvT[:, t0:t0 + SW], kvT[:, t0:t0 + SW])
            for (onev, sqt, invr, rk) in ((ones96_bf, qsq, inv_q_rows[s], Q_RANK),
                                          (ones32_bf, ksq, inv_kv_rows[s], KV_RANK)):
                ssq = psO.tile([1, SW], FP32, tag="ot")
                nc.tensor.matmul(ssq, onev, sqt, start=True, stop=True)
                mtmp = work.tile([1, SW], FP32, tag="mtmp")
                nc.vector.tensor_scalar(mtmp, ssq, 1.0 / rk, EPS,
                                        op0=OP.mult, op1=OP.add)
                stmp = work.tile([1, SW], FP32, tag="stmp")
                nc.scalar.activation(stmp, mtmp, AF.Sqrt)
                nc.vector.reciprocal(invr, stmp)
            # transpose kv inv to column form for V1 scaling
            for b in range(4):
                tp = psP.tile([128, 1], FP32, tag="pp")
                nc.tensor.transpose(tp, inv_kv_rows[s][:, b * 128:(b + 1) * 128],
                                    ident1)
                nc.scalar.copy(inv_colT[:, s * 4 + b:s * 4 + b + 1], tp)

        # ---- Vwin + gate/imp projection (heads paired, 2 blocks/psum) ----
        for b2 in range(TB // 2):
            ps = psO.tile([128, 2, 196], FP32, tag="ot", name=f"pvw{b2}")
            for j in range(2):
                b = b2 * 2 + j
                for cb in range(CB):
                    nc.tensor.matmul(ps[:, j, :],
                                     xT_sb[:, cb, b * 128:(b + 1) * 128],
                                     wwvgi_sb[:, cb, :],
                                     start=(cb == 0), stop=(cb == CB - 1))
            for j in range(2):
                b = b2 * 2 + j
                for h in range(2):
                    nc.scalar.copy(Vw[h][:, b, 0:VHD],
                                   ps[:, j, h * 96:h * 96 + 96])
                nc.vector.tensor_add(gi_acc, gi_acc, ps[:, j, 192:196])

        # gate = softmax(sum/T + b_gate)
        glp = psO.tile([1, 4], FP32, tag="ot")
        nc.tensor.matmul(glp, ones128_f32, gi_acc, start=True, stop=True)
        gl = work.tile([1, 4], FP32, tag="gl")
        nc.scalar.activation(gl, glp, AF.Copy, scale=1.0 / T)
        nc.vector.tensor_add(gl[0:1, 0:3], gl[0:1, 0:3], bgate_sb)
        ge = work.tile([1, 3], FP32, tag="ge")
        nc.scalar.activation(ge, gl[0:1, 0:3], AF.Exp)
        gs = work.tile([1, 1], FP32, tag="gs")
        nc.vector.reduce_sum(gs, ge, axis=mybir.AxisListType.X)
        gr = work.tile([1, 1], FP32, tag="gr")
        nc.vector.reciprocal(gr, gs)
        nc.vector.tensor_scalar_mul(gate_sb, ge, gr)
        gate_bc = sbA.tile([96, 3], FP32)
        nc.gpsimd.partition_broadcast(gate_bc, gate_sb)

        # ---- top-k gather: selT[p, cb, i] = x_bf[idx[i], cb*128+p] ----
        nc.gpsimd.dma_gather(
            out_ap=selT[:],
            in_ap=xbf_d.ap(),
            idxs_ap=idx_sb[:],
            num_idxs=S_KEEP,
            num_idxs_reg=S_KEEP,
            elem_size=C,
            transpose=True,
        )

        # ---- Vsel (heads paired, 2 blocks/psum) ----
        for b2 in range(2):
            ps = psO.tile([128, 2, 192], FP32, tag="ot", name=f"pvs{b2}")
            for j in range(2):
                b = b2 * 2 + j
                for cb in range(CB):
                    nc.tensor.matmul(ps[:, j, :],
                                     selT[:, cb, b * 128:(b + 1) * 128],
                                     wsv_sb[:, cb, :],
                                     start=(cb == 0), stop=(cb == CB - 1))
            for j in range(2):
                b = b2 * 2 + j
                for h in range(2):
                    nc.scalar.copy(Vs[h][:, b, 0:VHD],
                                   ps[:, j, h * 96:h * 96 + 96])

        # ---- V1 (heads paired, 2 blocks/psum, kv inv scaling at evict) ----
        for b2 in range(TB // 2):
            ps = psO.tile([128, 2, 192], FP32, tag="ot", name=f"pv1{b2}")
            for j in range(2):
                b = b2 * 2 + j
                nc.tensor.matmul(ps[:, j, :], kvT[:, b * 128:(b + 1) * 128],
                                 wdv_sb, start=True, stop=True)
            for j in range(2):
                b = b2 * 2 + j
                for h in range(2):
                    nc.vector.tensor_scalar_mul(V1[h][:, b, 0:VHD],
                                                ps[:, j, h * 96:h * 96 + 96],
                                                inv_colT[:, b:b + 1])

        def rope_from(ps, out_t, t0, cos_slc, sin_slc, inv_bc):
            """Evict [96, w] psum -> out_t cols t0:t0+w applying optional
            per-column inv scaling (inv_bc broadcast tile or None) + RoPE on
            rows 32:96."""
            w = cos_slc.shape[-1]
            if inv_bc is not None:
                nc.vector.tensor_mul(out_t[0:32, t0:t0 + w], ps[0:32, :],
                                     inv_bc[0:32, :])
                p1 = ropep.tile([32, SW], FP32, tag="p1")
                p2 = ropep.tile([32, SW], FP32, tag="p2")
                nc.vector.tensor_mul(p1[:, 0:w], ps[32:64, :], inv_bc[32:64, :])
                nc.vector.tensor_mul(p2[:, 0:w], ps[64:96, :], inv_bc[64:96, :])
                r, i = p1[:, 0:w], p2[:, 0:w]
            else:
                nc.scalar.copy(out_t[0:32, t0:t0 + w], ps[0:32, :])
                r, i = ps[32:64, :], ps[64:96, :]
            ta = ropep.tile([32, SW], FP32, tag="ta")
            tb = ropep.tile([32, SW], FP32, tag="tb")
            nc.vector.tensor_mul(ta[:, 0:w], r, cos_slc)
            nc.vector.tensor_mul(tb[:, 0:w], i, sin_slc)
            nc.vector.tensor_sub(out_t[32:64, t0:t0 + w], ta[:, 0:w], tb[:, 0:w])
            nc.vector.tensor_mul(ta[:, 0:w], r, sin_slc)
            nc.vector.tensor_mul(tb[:, 0:w], i, cos_slc)
            nc.vector.tensor_add(out_t[64:96, t0:t0 + w], ta[:, 0:w], tb[:, 0:w])

        # ---- per-head projections (both heads) ----
        def d_proj(h):
            sl = slice(h * 96, h * 96 + 96)
            for s2 in range(NS // 2):
                t0 = s2 * 2 * SW
                W2 = 2 * SW
                for (wt, src, out_t, ib) in (
                        (wdq_sb[:, sl], qnT, QT[h], ibq_f),
                        (wdk_sb[:, sl], kvT, K1T[h], ibk_f)):
                    stg = ropep.tile([Q_RANK, 2 * SW], BF16, tag="stg",
                                     name=f"stgd{h}{s2}")
                    for j in range(2):
                        ts_ = t0 + j * SW
                        ps = psP.tile([Q_RANK, SW], FP32, tag="pp",
                                      name=f"psd{h}{s2}{j}")
                        nc.tensor.matmul(ps, wt, src[:, ts_:ts_ + SW],
                                         start=True, stop=True)
                        nc.scalar.copy(stg[:, j * SW:(j + 1) * SW], ps)
                    rope_stg(stg, out_t, t0, W2, t0,
                             ib[:, t0:t0 + W2] if ib is not None else None)
                stg = ropep.tile([Q_RANK, 2 * SW], BF16, tag="stg", name=f"stgw{h}{s2}")
                for j in range(2):
                    ts_ = t0 + j * SW
                    ps = psP.tile([Q_RANK, SW], FP32, tag="pp", name=f"psw{h}{s2}{j}")
                    for cb in range(CB):
                        nc.tensor.matmul(ps, wwk_sb[:, cb, sl],
                                         xT_sb[:, cb, ts_:ts_ + SW],
                                         start=(cb == 0), stop=(cb == CB - 1))
                    nc.scalar.copy(stg[:, j * SW:(j + 1) * SW], ps)
                rope_stg(stg, KwT[h], t0, W2, t0, None)

            # Ksel^T (rope positions 0..S_KEEP-1)
            stg = ropep.tile([Q_RANK, 2 * SW], BF16, tag="stg", name=f"stgs{h}")
            ps = psP.tile([Q_RANK, S_KEEP], FP32, tag="pp", name=f"pss{h}")
            for cb in range(CB):
                nc.tensor.matmul(ps, wsk_sb[:, cb, sl], selT[:, cb, :],
                                 start=(cb == 0), stop=(cb == CB - 1))
            nc.scalar.copy(stg[:, 0:S_KEEP], ps)
            rope_stg(stg, KsT[h], 0, S_KEEP, 0, None)

        # ---- attention per head (projections emitted just before each) ----
        for h in range(2):
            d_proj(h)
            # attention: 3 branches per strip
            for s in range(NS):
                t0 = s * SW
                qs = QT[h][:, t0:t0 + SW]
                otacc = None
                for br, (KT, Vt, causal) in enumerate([
                        (K1T[h], V1[h], True),
                        (KsT[h], Vs[h], False),
                        (KwT[h], Vw[h], True)]):
                    nk = 4 * (s + 1) if causal else 4
                    otp_ps = psO.tile([VA, SW], FP32, tag="ot")
                    for pair in range(nk // 2):
                        # diagonal key-blocks only need columns >= c0 (keys can
                        # never attend to earlier queries); the rest is masked
                        def col0(kb):
                            return (max(0, kb * 128 - s * SW)
                                    if causal and kb >= nk - 4 else 0)
                        kbs = (pair * 2, pair * 2 + 1)
                        c0s = [col0(kb) for kb in kbs]
                        cu = min(c0s)
                        sp = psS.tile([128, 2, SW], FP32, tag="sp")
                        for j, kb in enumerate(kbs):
                            nc.tensor.matmul(sp[:, j, c0s[j]:SW],
                                             KT[:, kb * 128:(kb + 1) * 128],
                                             qs[:, c0s[j]:SW],
                                             start=True, stop=True)
                        es = expp.tile([128, 2, SW], BF16, tag="es")
                        if c0s[0] == c0s[1]:
                            nc.scalar.activation(es[:, :, cu:SW], sp[:, :, cu:SW],
                                                 AF.Exp, scale=SCALE)
                        else:
                            for j in range(2):
                                nc.scalar.activation(es[:, j, c0s[j]:SW],
                                                     sp[:, j, c0s[j]:SW],
                                                     AF.Exp, scale=SCALE)
                        for j, kb in enumerate(kbs):
                            c0 = c0s[j]
                            if causal and kb >= nk - 4:
                                # partial triangle lives in the first 128 cols
                                nc.vector.tensor_mul(es[:, j, c0:c0 + 128],
                                                     es[:, j, c0:c0 + 128],
                                                     mask_sb[:, 0:128])
                            nc.tensor.matmul(otp_ps[:, c0:SW], Vt[:, kb, :],
                                             es[:, j, c0:SW],
                                             start=(kb == 0), stop=(kb == nk - 1))
                    rec = recp.tile([1, SW], FP32, tag="rec")
                    nc.vector.reciprocal(rec, otp_ps[VHD:VA, :])
                    obc = bcp.tile([96, SW], FP32, tag="obc")
                    nc.gpsimd.partition_broadcast(obc, rec)
                    g = gate_bc[:, br:br + 1]
                    if br == 0:
                        otacc = otp.tile([96, SW], FP32, tag="otacc")
                        nc.vector.scalar_tensor_tensor(
                            otacc, in0=otp_ps[0:VHD, :], scalar=g, in1=obc,
                            op0=OP.mult, op1=OP.mult)
                    else:
                        tmp = otp.tile([96, SW], FP32, tag="otmp")
                        nc.vector.scalar_tensor_tensor(
                            tmp, in0=otp_ps[0:VHD, :], scalar=g, in1=obc,
                            op0=OP.mult, op1=OP.mult)
                        dst = OTt[(h, s)] if br == 2 else otacc
                        nc.vector.tensor_add(dst, otacc, tmp)

        # ---- out-projection (partial y on the 2-head slice) ----
        for b in range(TB):
            s = b // 4
            c0 = (b % 4) * 128
            yt = ysb.tile([128, C], FP32, tag="yt")
            for half in range(2):
                yp = psP.tile([128, SW], FP32, tag="pp")
                for h in range(2):
                    nc.tensor.matmul(yp, OTt[(h, s)][:, c0:c0 + 128],
                                     wproj_sb[:, h * C + half * SW:
                                              h * C + half * SW + SW],
                                     start=(h == 0), stop=(h == 1))
                nc.vector.tensor_copy(yt[:, half * SW:half * SW + SW], yp)
            nc.sync.dma_start(y_d.ap()[b * 128:(b + 1) * 128, :], yt)


# ------------------------------------------------------------------
# host side
# ------------------------------------------------------------------

_BUILT = None


def _build():
    global _BUILT
    if _BUILT is None:
        nc = bacc.Bacc(None, target_bir_lowering=False, debug=False)
        _emit(nc)
        nc.finalize()
        _BUILT = nc
    return _BUILT


def _bf(x):
    return np.ascontiguousarray(np.asarray(x, np.float32)).astype(bf16)


def host_prep(inputs):
    inp = {k: np.asarray(v, np.float32) if np.asarray(v).dtype == np.float32
           else np.asarray(v) for k, v in inputs.items()}
    x = np.asarray(inp['x'], np.float32)[0]          # [T, C]
    shared = {}
    shared['xT'] = _bf(x.T)
    shared['x_bf'] = _bf(x)
    inv = 1.0 / (10000.0 ** (np.arange(0, ROPE_D, 2, dtype=np.float32) / ROPE_D))
    f = np.outer(inv, np.arange(T, dtype=np.float32))
    shared['cos_t'] = np.cos(f).astype(bf16)
    shared['sin_t'] = np.sin(f).astype(bf16)
    k = np.arange(128)[:, None]
    cgrid = np.arange(128)[None, :]
    shared['mask_strip'] = (k <= cgrid).astype(bf16)
    scores = x @ np.asarray(inp['w_imp'], np.float32)[:, 0]
    idx = np.sort(np.argsort(-scores)[:S_KEEP])
    wrapped = idx.astype(np.int16).reshape(S_KEEP // 16, 16).T
    shared['idx16'] = np.ascontiguousarray(np.tile(wrapped, (8, 1)).astype(np.int16))
    shared['Wqkv'] = _bf(np.concatenate([inp['w_cq'], inp['w_ckv']], axis=1))
    shared['bgate'] = np.asarray(inp['b_gate'], np.float32)[None, :]

    qs = np.asarray(inp['q_scale'], np.float32)[:, None]
    kvs = np.asarray(inp['kv_scale'], np.float32)[:, None]
    maps = []
    for c in range(N_CORES):
        hs = [2 * c, 2 * c + 1]
        m = dict(shared)
        m['Wdq'] = _bf(np.concatenate(
            [qs * np.concatenate(
                [inp['w_dq_nope'][:, h * NOPE:(h + 1) * NOPE],
                 inp['w_dq_rope'][:, h * ROPE_D:(h + 1) * ROPE_D]], axis=1)
             for h in hs], axis=1))
        m['Wdk'] = _bf(np.concatenate(
            [kvs * np.concatenate(
                [inp['w_dk_nope'][:, h * NOPE:(h + 1) * NOPE],
                 inp['w_k_rope'][:, h * ROPE_D:(h + 1) * ROPE_D]], axis=1)
             for h in hs], axis=1))
        m['Wdv'] = _bf(np.concatenate(
            [kvs * inp['w_dv'][:, h * VHD:(h + 1) * VHD] for h in hs], axis=1))
        m['Wwk'] = _bf(np.concatenate(
            [inp['w_win_k'][:, h * HEAD_D:(h + 1) * HEAD_D] for h in hs], axis=1))
        m['Wsk'] = _bf(np.concatenate(
            [inp['w_sel_k'][:, h * HEAD_D:(h + 1) * HEAD_D] for h in hs], axis=1))
        m['Wwv_gi'] = _bf(np.concatenate(
            [inp['w_win_v'][:, h * VHD:(h + 1) * VHD] for h in hs]
            + [inp['w_gate'], inp['w_imp']], axis=1))
        m['Wsv'] = _bf(np.concatenate(
            [inp['w_sel_v'][:, h * VHD:(h + 1) * VHD] for h in hs], axis=1))
        m['Wproj'] = _bf(np.concatenate(
            [inp['w_proj'][h * VHD:(h + 1) * VHD, :] for h in hs], axis=1))
        maps.append(m)
    return maps


def run(inputs, **kw):
    nc = _build()
    in_maps = host_prep(inputs)
    res = bass_utils.run_bass_kernel_spmd(nc, in_maps, core_ids=list(range(N_CORES)),
                                          **kw)
    y = np.zeros((T, C), np.float32)
    for r in res.results:
        y += r['y']
    return y[None].astype(np.float32), res


def kernel(**inputs):
    y, _ = run(inputs)
    return y


# revision 71
# speedup vs baseline: 1.2153x; 1.0033x over previous
"""Trainium2 Bass kernel for nn_Attn_33028298506245 (sparse MLA-style attention).

Sharding: tensor-parallel over the 16 heads -> 2 heads per NeuronCore (8 cores).
Shared work (compressed q/kv projection, rmsnorm stats, gate, token top-k
selection gather) is replicated per core; the final out-projection is computed
per-core on that core's head slice and the partial [T, C] outputs are summed on
the host (the all-reduce / unshard step).

Device algorithm (per core, transposed "d-on-partitions" orientation):
  - qkv_nT = [w_cq|w_ckv]^T x^T               [128, T]  (rows 0:96 q, 96:128 kv)
  - rms stats via ones-mask matmul; inv scales folded into Q^T/K^T columns
  - Q^T/K^T produced directly transposed; RoPE applied in [d, T] layout
  - attention computed as S^T = K^T_block^T Q^T  [keys, queries]; softmax along
    the key (partition) axis via exp + ones-row-augmented V matmul (denominator
    comes out as row 96 of the PV accumulation); causal masking via a
    precomputed staircase strip multiply on the 4 diagonal key-blocks
  - branch 2 uses dma_gather (transpose=True) to build sel^T from the top-512
    tokens; rope positions are 0..511 as in the reference
  - gated combine of the 3 branches, then out-proj on the head slice
"""
import os
import sys

for _p in ("/opt/trn_rl_repo", "/root/.axon_site/_ro/trn_rl_repo"):
    if os.path.isdir(_p) and _p not in sys.path:
        sys.path.append(_p)

import numpy as np
import ml_dtypes

import concourse.mybir as mybir
import concourse.tile as tile
from concourse import bacc
from concourse import bass_utils

bf16 = ml_dtypes.bfloat16
FP32 = mybir.dt.float32
BF16 = mybir.dt.bfloat16

B, T, C = 1, 2048, 1024
H = 16
NOPE, ROPE_D, VHD = 32, 64, 96
KV_RANK, Q_RANK = 32, 96
S_KEEP = 512
EPS = 1e-6
HEAD_D = NOPE + ROPE_D          # 96
SCALE = 1.0 / float(np.sqrt(HEAD_D))
N_CORES = 8
NS = 4                          # strips of 512 queries
SW = 512                        # strip width
TB = 16                         # 128-token blocks
CB = 8                          # 128-channel blocks
VA = VHD + 1                    # V augmented with ones row -> denominator

AF = mybir.ActivationFunctionType
OP = mybir.AluOpType


def _emit(nc):
    dt_in = {}

    def din(name, shape, dtype):
        t = nc.dram_tensor(name, shape, dtype, kind="ExternalInput")
        dt_in[name] = t
        return t

    xT_d = din("xT", [C, T], BF16)
    xbf_d = din("x_bf", [T, C], BF16)
    cos_d = din("cos_t", [32, T], BF16)
    sin_d = din("sin_t", [32, T], BF16)
    mask_d = din("mask_strip", [128, 128], BF16)
    idx_d = din("idx16", [128, S_KEEP // 16], mybir.dt.int16)
    wqkv_d = din("Wqkv", [C, 128], BF16)
    wdq_d = din("Wdq", [Q_RANK, 192], BF16)
    wdk_d = din("Wdk", [KV_RANK, 192], BF16)
    wdv_d = din("Wdv", [KV_RANK, 192], BF16)
    wwk_d = din("Wwk", [C, 192], BF16)
    wsk_d = din("Wsk", [C, 192], BF16)
    wwvgi_d = din("Wwv_gi", [C, 196], BF16)
    wsv_d = din("Wsv", [C, 192], BF16)
    wproj_d = din("Wproj", [Q_RANK, 2 * C], BF16)
    bgate_d = din("bgate", [1, 3], FP32)
    y_d = nc.dram_tensor("y", [T, C], FP32, kind="ExternalOutput")

    with tile.TileContext(nc) as tc:
        _body(nc, tc, xT_d, xbf_d, cos_d, sin_d, mask_d, idx_d, wqkv_d, wdq_d,
              wdk_d, wdv_d, wwk_d, wsk_d, wwvgi_d, wsv_d, wproj_d, bgate_d, y_d)
    return dt_in


def _body(nc, tc, xT_d, xbf_d, cos_d, sin_d, mask_d, idx_d, wqkv_d, wdq_d,
          wdk_d, wdv_d, wwk_d, wsk_d, wwvgi_d, wsv_d, wproj_d, bgate_d, y_d):
    from contextlib import ExitStack
    ctx = ExitStack()
    with ctx:
        const = ctx.enter_context(tc.tile_pool(name="const", bufs=1))
        sbA = ctx.enter_context(tc.tile_pool(name="sbA", bufs=1))
        work = ctx.enter_context(tc.tile_pool(name="work", bufs=2))
        ropep = ctx.enter_context(tc.tile_pool(name="ropep", bufs=3))
        expp = ctx.enter_context(tc.tile_pool(name="expp", bufs=5))
        bcp = ctx.enter_context(tc.tile_pool(name="bcp", bufs=2))
        recp = ctx.enter_context(tc.tile_pool(name="recp", bufs=4))
        otp = ctx.enter_context(tc.tile_pool(name="otp", bufs=2))
        ysb = ctx.enter_context(tc.tile_pool(name="ysb", bufs=3))
        psS = ctx.enter_context(tc.tile_pool(name="psS", bufs=2, space="PSUM"))
        psO = ctx.enter_context(tc.tile_pool(name="psO", bufs=2, space="PSUM"))
        psP = ctx.enter_context(tc.tile_pool(name="psP", bufs=2, space="PSUM"))

        # ---- constants / weights into SBUF ----
        # weights needed first come first; x^T is loaded strip-major in the
        # order the A-phase consumes it so the first matmul starts early
        wqkv_sb = const.tile([128, CB, 128], BF16)
        nc.sync.dma_start(wqkv_sb, wqkv_d.ap().rearrange("(cb p) m -> p cb m", p=128))
        xT_sb = const.tile([128, CB, T], BF16)
        for st in range(NS):
            for cb in range(CB):
                nc.sync.dma_start(xT_sb[:, cb, st * SW:(st + 1) * SW],
                                  xT_d.ap()[cb * 128:(cb + 1) * 128,
                                            st * SW:(st + 1) * SW])
        cos_sb = const.tile([32, T], FP32)
        sin_sb = const.tile([32, T], FP32)
        nc.sync.dma_start(cos_sb, cos_d.ap())
        nc.sync.dma_start(sin_sb, sin_d.ap())
        mask_sb = const.tile([128, 128], BF16)
        nc.sync.dma_start(mask_sb, mask_d.ap())
        idx_sb = const.tile([128, S_KEEP // 16], mybir.dt.int16)
        nc.sync.dma_start(idx_sb, idx_d.ap())

        wdq_sb = const.tile([Q_RANK, 192], BF16)
        nc.sync.dma_start(wdq_sb, wdq_d.ap())
        wdk_sb = const.tile([KV_RANK, 192], BF16)
        nc.sync.dma_start(wdk_sb, wdk_d.ap())
        wdv_sb = const.tile([KV_RANK, 192], BF16)
        nc.sync.dma_start(wdv_sb, wdv_d.ap())
        wwk_sb = const.tile([128, CB, 192], BF16)
        nc.sync.dma_start(wwk_sb, wwk_d.ap().rearrange("(cb p) m -> p cb m", p=128))
        wsk_sb = const.tile([128, CB, 192], BF16)
        nc.sync.dma_start(wsk_sb, wsk_d.ap().rearrange("(cb p) m -> p cb m", p=128))
        wwvgi_sb = const.tile([128, CB, 196], BF16)
        nc.sync.dma_start(wwvgi_sb, wwvgi_d.ap().rearrange("(cb p) m -> p cb m", p=128))
        wsv_sb = const.tile([128, CB, 192], BF16)
        nc.sync.dma_start(wsv_sb, wsv_d.ap().rearrange("(cb p) m -> p cb m", p=128))
        wproj_sb = const.tile([Q_RANK, 2 * C], BF16)
        nc.sync.dma_start(wproj_sb, wproj_d.ap())
        bgate_sb = const.tile([1, 3], FP32)
        nc.sync.dma_start(bgate_sb, bgate_d.ap())

        ones96_bf = const.tile([Q_RANK, 1], BF16)
        nc.vector.memset(ones96_bf, 1.0)
        ones32_bf = const.tile([KV_RANK, 1], BF16)
        nc.vector.memset(ones32_bf, 1.0)
        ones128_f32 = const.tile([128, 1], FP32)
        nc.vector.memset(ones128_f32, 1.0)
        ident1 = const.tile([1, 1], FP32)
        nc.vector.memset(ident1, 1.0)

        # ---- persistent intermediates ----
        qnT = sbA.tile([Q_RANK, T], BF16)               # q_nT (unnormalized)
        kvT = sbA.tile([KV_RANK, T], BF16)              # kv_nT (unnormalized)
        selT = sbA.tile([128, CB, S_KEEP], BF16)        # sel^T gathered
        inv_q_rows = [sbA.tile([1, SW], FP32, tag=f"invq{s}", name=f"invq{s}") for s in range(NS)]
        inv_kv_rows = [sbA.tile([1, SW], FP32, tag=f"invk{s}", name=f"invk{s}") for s in range(NS)]
        inv_colT = sbA.tile([128, TB], FP32)            # per-token kv inv (columns)
        gi_acc = sbA.tile([128, 4], FP32)
        gate_sb = sbA.tile([1, 3], FP32)
        QT = [sbA.tile([Q_RANK, T], BF16, tag=f"QT{h}", name=f"QT{h}") for h in range(2)]
        K1T = [sbA.tile([Q_RANK, T], BF16, tag=f"K1T{h}", name=f"K1T{h}") for h in range(2)]
        KwT = [sbA.tile([Q_RANK, T], BF16, tag=f"KwT{h}", name=f"KwT{h}") for h in range(2)]
        KsT = [sbA.tile([Q_RANK, S_KEEP], BF16, tag=f"KsT{h}", name=f"KsT{h}") for h in range(2)]
        V1 = [sbA.tile([128, TB, VA], BF16, tag=f"V1{h}", name=f"V1_{h}") for h in range(2)]
        Vw = [sbA.tile([128, TB, VA], BF16, tag=f"Vw{h}", name=f"Vw_{h}") for h in range(2)]
        Vs = [sbA.tile([128, 4, VA], BF16, tag=f"Vs{h}", name=f"Vs_{h}") for h in range(2)]
        OTt = {(h, s): sbA.tile([Q_RANK, SW], BF16, tag=f"OTt{h}_{s}", name=f"OTt{h}_{s}")
               for h in range(2) for s in range(NS)}

        # ones rows of augmented V (never overwritten afterwards)
        for h in range(2):
            nc.vector.memset(V1[h][:, :, VHD:VA], 1.0)
            nc.vector.memset(Vw[h][:, :, VHD:VA], 1.0)
            nc.vector.memset(Vs[h][:, :, VHD:VA], 1.0)
        nc.vector.memset(gi_acc, 0.0)

        # ---- A: shared projection + rms stats per strip ----
        for s in range(NS):
            t0 = s * SW
            ps = psP.tile([128, SW], FP32, tag="pp")
            for cb in range(CB):
                nc.tensor.matmul(ps, wqkv_sb[:, cb, :], xT_sb[:, cb, t0:t0 + SW],
                                 start=(cb == 0), stop=(cb == CB - 1))
            nc.scalar.copy(qnT[:, t0:t0 + SW], ps[0:Q_RANK, :])
            # 32-partition quadrant move (q3 -> q0) on DVE
            nc.vector.tensor_copy(kvT[:, t0:t0 + SW], ps[Q_RANK:128, :])
            qsq = work.tile([Q_RANK, SW], BF16, tag="qsq")
            nc.vector.tensor_mul(qsq, qnT[:, t0:t0 + SW], qnT[:, t0:t0 + SW])
            ksq = work.tile([KV_RANK, SW], BF16, tag="ksq")
            nc.vector.tensor_mul(ksq, kvT[:, t0:t0 + SW], kvT[:, t0:t0 + SW])
            for (onev, sqt, invr, rk) in ((ones96_bf, qsq, inv_q_rows[s], Q_RANK),
                                          (ones32_bf, ksq, inv_kv_rows[s], KV_RANK)):
                ssq = psO.tile([1, SW], FP32, tag="ot")
                nc.tensor.matmul(ssq, onev, sqt, start=True, stop=True)
                mtmp = work.tile([1, SW], FP32, tag="mtmp")
                nc.vector.tensor_scalar(mtmp, ssq, 1.0 / rk, EPS,
                                        op0=OP.mult, op1=OP.add)
                stmp = work.tile([1, SW], FP32, tag="stmp")
                nc.scalar.activation(stmp, mtmp, AF.Sqrt)
                nc.vector.reciprocal(invr, stmp)
            # transpose kv inv to column form for V1 scaling
            for b in range(4):
                tp = psP.tile([128, 1], FP32, tag="pp")
                nc.tensor.transpose(tp, inv_kv_rows[s][:, b * 128:(b + 1) * 128],
                                    ident1)
                nc.scalar.copy(inv_colT[:, s * 4 + b:s * 4 + b + 1], tp)

        # ---- Vwin + gate/imp projection (heads paired, 2 blocks/psum) ----
        for b2 in range(TB // 2):
            ps = psO.tile([128, 2, 196], FP32, tag="ot", name=f"pvw{b2}")
            for j in range(2):
                b = b2 * 2 + j
                for cb in range(CB):
                    nc.tensor.matmul(ps[:, j, :],
                                     xT_sb[:, cb, b * 128:(b + 1) * 128],
                                     wwvgi_sb[:, cb, :],
                                     start=(cb == 0), stop=(cb == CB - 1))
            for j in range(2):
                b = b2 * 2 + j
                for h in range(2):
                    nc.scalar.copy(Vw[h][:, b, 0:VHD],
                                   ps[:, j, h * 96:h * 96 + 96])
                nc.vector.tensor_add(gi_acc, gi_acc, ps[:, j, 192:196])

        # gate = softmax(sum/T + b_gate)
        glp = psO.tile([1, 4], FP32, tag="ot")
        nc.tensor.matmul(glp, ones128_f32, gi_acc, start=True, stop=True)
        gl = work.tile([1, 4], FP32, tag="gl")
        nc.scalar.activation(gl, glp, AF.Copy, scale=1.0 / T)
        nc.vector.tensor_add(gl[0:1, 0:3], gl[0:1, 0:3], bgate_sb)
        ge = work.tile([1, 3], FP32, tag="ge")
        nc.scalar.activation(ge, gl[0:1, 0:3], AF.Exp)
        gs = work.tile([1, 1], FP32, tag="gs")
        nc.vector.reduce_sum(gs, ge, axis=mybir.AxisListType.X)
        gr = work.tile([1, 1], FP32, tag="gr")
        nc.vector.reciprocal(gr, gs)
        nc.vector.tensor_scalar_mul(gate_sb, ge, gr)
        gate_bc = sbA.tile([96, 3], FP32)
        nc.gpsimd.partition_broadcast(gate_bc, gate_sb)

        # ---- top-k gather: selT[p, cb, i] = x_bf[idx[i], cb*128+p] ----
        nc.gpsimd.dma_gather(
            out_ap=selT[:],
            in_ap=xbf_d.ap(),
            idxs_ap=idx_sb[:],
            num_idxs=S_KEEP,
            num_idxs_reg=S_KEEP,
            elem_size=C,
            transpose=True,
        )

        # ---- Vsel (heads paired, 2 blocks/psum) ----
        for b2 in range(2):
            ps = psO.tile([128, 2, 192], FP32, tag="ot", name=f"pvs{b2}")
            for j in range(2):
                b = b2 * 2 + j
                for cb in range(CB):
                    nc.tensor.matmul(ps[:, j, :],
                                     selT[:, cb, b * 128:(b + 1) * 128],
                                     wsv_sb[:, cb, :],
                                     start=(cb == 0), stop=(cb == CB - 1))
            for j in range(2):
                b = b2 * 2 + j
                for h in range(2):
                    nc.scalar.copy(Vs[h][:, b, 0:VHD],
                                   ps[:, j, h * 96:h * 96 + 96])

        # ---- V1 (heads paired, 2 blocks/psum, kv inv scaling at evict) ----
        for b2 in range(TB // 2):
            ps = psO.tile([128, 2, 192], FP32, tag="ot", name=f"pv1{b2}")
            for j in range(2):
                b = b2 * 2 + j
                nc.tensor.matmul(ps[:, j, :], kvT[:, b * 128:(b + 1) * 128],
                                 wdv_sb, start=True, stop=True)
            for j in range(2):
                b = b2 * 2 + j
                for h in range(2):
                    nc.vector.tensor_scalar_mul(V1[h][:, b, 0:VHD],
                                                ps[:, j, h * 96:h * 96 + 96],
                                                inv_colT[:, b:b + 1])

        def rope_from(ps, out_t, t0, cos_slc, sin_slc, inv_bc):
            """Evict [96, w] psum -> out_t cols t0:t0+w applying optional
            per-column inv scaling (inv_bc broadcast tile or None) + RoPE on
            rows 32:96."""
            w = cos_slc.shape[-1]
            if inv_bc is not None:
                nc.vector.tensor_mul(out_t[0:32, t0:t0 + w], ps[0:32, :],
                                     inv_bc[0:32, :])
                p1 = ropep.tile([32, SW], FP32, tag="p1")
                p2 = ropep.tile([32, SW], FP32, tag="p2")
                nc.vector.tensor_mul(p1[:, 0:w], ps[32:64, :], inv_bc[32:64, :])
                nc.vector.tensor_mul(p2[:, 0:w], ps[64:96, :], inv_bc[64:96, :])
                r, i = p1[:, 0:w], p2[:, 0:w]
            else:
                nc.scalar.copy(out_t[0:32, t0:t0 + w], ps[0:32, :])
                r, i = ps[32:64, :], ps[64:96, :]
            ta = ropep.tile([32, SW], FP32, tag="ta")
            tb = ropep.tile([32, SW], FP32, tag="tb")
            nc.vector.tensor_mul(ta[:, 0:w], r, cos_slc)
            nc.vector.tensor_mul(tb[:, 0:w], i, sin_slc)
            nc.vector.tensor_sub(out_t[32:64, t0:t0 + w], ta[:, 0:w], tb[:, 0:w])
            nc.vector.tensor_mul(ta[:, 0:w], r, sin_slc)
            nc.vector.tensor_mul(tb[:, 0:w], i, cos_slc)
            nc.vector.tensor_add(out_t[64:96, t0:t0 + w], ta[:, 0:w], tb[:, 0:w])

        # ---- per-head projections (both heads) ----
        def d_proj(h):
            sl = slice(h * 96, h * 96 + 96)
            for s2 in range(NS // 2):
                t0 = s2 * 2 * SW
                W2 = 2 * SW
                for (wt, src, out_t, ib) in (
                        (wdq_sb[:, sl], qnT, QT[h], ibq_f),
                        (wdk_sb[:, sl], kvT, K1T[h], ibk_f)):
                    stg = ropep.tile([Q_RANK, 2 * SW], BF16, tag="stg",
                                     name=f"stgd{h}{s2}")
                    for j in range(2):
                        ts_ = t0 + j * SW
                        ps = psP.tile([Q_RANK, SW], FP32, tag="pp",
                                      name=f"psd{h}{s2}{j}")
                        nc.tensor.matmul(ps, wt, src[:, ts_:ts_ + SW],
                                         start=True, stop=True)
                        nc.scalar.copy(stg[:, j * SW:(j + 1) * SW], ps)
                    rope_stg(stg, out_t, t0, W2, t0,
                             ib[:, t0:t0 + W2] if ib is not None else None)
                stg = ropep.tile([Q_RANK, 2 * SW], BF16, tag="stg", name=f"stgw{h}{s2}")
                for j in range(2):
                    ts_ = t0 + j * SW
                    ps = psP.tile([Q_RANK, SW], FP32, tag="pp", name=f"psw{h}{s2}{j}")
                    for cb in range(CB):
                        nc.tensor.matmul(ps, wwk_sb[:, cb, sl],
                                         xT_sb[:, cb, ts_:ts_ + SW],
                                         start=(cb == 0), stop=(cb == CB - 1))
                    nc.scalar.copy(stg[:, j * SW:(j + 1) * SW], ps)
                rope_stg(stg, KwT[h], t0, W2, t0, None)

            # Ksel^T (rope positions 0..S_KEEP-1)
            stg = ropep.tile([Q_RANK, 2 * SW], BF16, tag="stg", name=f"stgs{h}")
            ps = psP.tile([Q_RANK, S_KEEP], FP32, tag="pp", name=f"pss{h}")
            for cb in range(CB):
                nc.tensor.matmul(ps, wsk_sb[:, cb, sl], selT[:, cb, :],
                                 start=(cb == 0), stop=(cb == CB - 1))
            nc.scalar.copy(stg[:, 0:S_KEEP], ps)
            rope_stg(stg, KsT[h], 0, S_KEEP, 0, None)

        # ---- attention per head (projections emitted just before each) ----
        for h in range(2):
            d_proj(h)
            # attention: 3 branches per strip
            for s in range(NS):
                t0 = s * SW
                qs = QT[h][:, t0:t0 + SW]
                otacc = None
                for br, (KT, Vt, causal) in enumerate([
                        (K1T[h], V1[h], True),
                        (KsT[h], Vs[h], False),
                        (KwT[h], Vw[h], True)]):
                    nk = 4 * (s + 1) if causal else 4
                    otp_ps = psO.tile([VA, SW], FP32, tag="ot")
                    for pair in range(nk // 2):
                        # diagonal key-blocks only need columns >= c0 (keys can
                        # never attend to earlier queries); the rest is masked
                        def col0(kb):
                            return (max(0, kb * 128 - s * SW)
                                    if causal and kb >= nk - 4 else 0)
                        kbs = (pair * 2, pair * 2 + 1)
                        c0s = [col0(kb) for kb in kbs]
                        cu = min(c0s)
                        sp = psS.tile([128, 2, SW], FP32, tag="sp")
                        for j, kb in enumerate(kbs):
                            nc.tensor.matmul(sp[:, j, c0s[j]:SW],
                                             KT[:, kb * 128:(kb + 1) * 128],
                                             qs[:, c0s[j]:SW],
                                             start=True, stop=True)
                        es = expp.tile([128, 2, SW], BF16, tag="es")
                        if c0s[0] == c0s[1]:
                            nc.scalar.activation(es[:, :, cu:SW], sp[:, :, cu:SW],
                                                 AF.Exp, scale=SCALE)
                        else:
                            for j in range(2):
                                nc.scalar.activation(es[:, j, c0s[j]:SW],
                                                     sp[:, j, c0s[j]:SW],
                                                     AF.Exp, scale=SCALE)
                        for j, kb in enumerate(kbs):
                            c0 = c0s[j]
                            if causal and kb >= nk - 4:
                                # partial triangle lives in the first 128 cols
                                nc.vector.tensor_mul(es[:, j, c0:c0 + 128],
                                                     es[:, j, c0:c0 + 128],
                                                     mask_sb[:, 0:128])
                            nc.tensor.matmul(otp_ps[:, c0:SW], Vt[:, kb, :],
                                             es[:, j, c0:SW],
                                             start=(kb == 0), stop=(kb == nk - 1))
                    rec = recp.tile([1, SW], FP32, tag="rec")
                    nc.vector.reciprocal(rec, otp_ps[VHD:VA, :])
                    obc = bcp.tile([96, SW], FP32, tag="obc")
                    nc.gpsimd.partition_broadcast(obc, rec)
                    g = gate_bc[:, br:br + 1]
                    if br == 0:
                        otacc = otp.tile([96, SW], FP32, tag="otacc")
                        nc.vector.scalar_tensor_tensor(
                            otacc, in0=otp_ps[0:VHD, :], scalar=g, in1=obc,
                            op0=OP.mult, op1=OP.mult)
                    else:
                        tmp = otp.tile([96, SW], FP32, tag="otmp")
                        nc.vector.scalar_tensor_tensor(
                            tmp, in0=otp_ps[0:VHD, :], scalar=g, in1=obc,
                            op0=OP.mult, op1=OP.mult)
                        dst = OTt[(h, s)] if br == 2 else otacc
                        nc.vector.tensor_add(dst, otacc, tmp)

        # ---- out-projection (partial y on the 2-head slice) ----
        for b in range(TB):
            s = b // 4
            c0 = (b % 4) * 128
            yt = ysb.tile([128, C], FP32, tag="yt")
            for half in range(2):
                yp = psP.tile([128, SW], FP32, tag="pp")
                for h in range(2):
                    nc.tensor.matmul(yp, OTt[(h, s)][:, c0:c0 + 128],
                                     wproj_sb[:, h * C + half * SW:
                                              h * C + half * SW + SW],
                                     start=(h == 0), stop=(h == 1))
                nc.vector.tensor_copy(yt[:, half * SW:half * SW + SW], yp)
                nc.sync.dma_start(
                    y_d.ap()[b * 128:(b + 1) * 128,
                             half * SW:half * SW + SW],
                    yt[:, half * SW:half * SW + SW])


# ------------------------------------------------------------------
# host side
# ------------------------------------------------------------------

_BUILT = None


def _build():
    global _BUILT
    if _BUILT is None:
        nc = bacc.Bacc(None, target_bir_lowering=False, debug=False)
        _emit(nc)
        nc.finalize()
        _BUILT = nc
    return _BUILT


def _bf(x):
    return np.ascontiguousarray(np.asarray(x, np.float32)).astype(bf16)


def host_prep(inputs):
    inp = {k: np.asarray(v, np.float32) if np.asarray(v).dtype == np.float32
           else np.asarray(v) for k, v in inputs.items()}
    x = np.asarray(inp['x'], np.float32)[0]          # [T, C]
    shared = {}
    shared['xT'] = _bf(x.T)
    shared['x_bf'] = _bf(x)
    inv = 1.0 / (10000.0 ** (np.arange(0, ROPE_D, 2, dtype=np.float32) / ROPE_D))
    f = np.outer(inv, np.arange(T, dtype=np.float32))
    shared['cos_t'] = np.cos(f).astype(bf16)
    shared['sin_t'] = np.sin(f).astype(bf16)
    k = np.arange(128)[:, None]
    cgrid = np.arange(128)[None, :]
    shared['mask_strip'] = (k <= cgrid).astype(bf16)
    scores = x @ np.asarray(inp['w_imp'], np.float32)[:, 0]
    idx = np.sort(np.argsort(-scores)[:S_KEEP])
    wrapped = idx.astype(np.int16).reshape(S_KEEP // 16, 16).T
    shared['idx16'] = np.ascontiguousarray(np.tile(wrapped, (8, 1)).astype(np.int16))
    shared['Wqkv'] = _bf(np.concatenate([inp['w_cq'], inp['w_ckv']], axis=1))
    shared['bgate'] = np.asarray(inp['b_gate'], np.float32)[None, :]

    qs = np.asarray(inp['q_scale'], np.float32)[:, None]
    kvs = np.asarray(inp['kv_scale'], np.float32)[:, None]
    maps = []
    for c in range(N_CORES):
        hs = [2 * c, 2 * c + 1]
        m = dict(shared)
        m['Wdq'] = _bf(np.concatenate(
            [qs * np.concatenate(
                [inp['w_dq_nope'][:, h * NOPE:(h + 1) * NOPE],
                 inp['w_dq_rope'][:, h * ROPE_D:(h + 1) * ROPE_D]], axis=1)
             for h in hs], axis=1))
        m['Wdk'] = _bf(np.concatenate(
            [kvs * np.concatenate(
                [inp['w_dk_nope'][:, h * NOPE:(h + 1) * NOPE],
                 inp['w_k_rope'][:, h * ROPE_D:(h + 1) * ROPE_D]], axis=1)
             for h in hs], axis=1))
        m['Wdv'] = _bf(np.concatenate(
            [kvs * inp['w_dv'][:, h * VHD:(h + 1) * VHD] for h in hs], axis=1))
        m['Wwk'] = _bf(np.concatenate(
            [inp['w_win_k'][:, h * HEAD_D:(h + 1) * HEAD_D] for h in hs], axis=1))
        m['Wsk'] = _bf(np.concatenate(
            [inp['w_sel_k'][:, h * HEAD_D:(h + 1) * HEAD_D] for h in hs], axis=1))
        m['Wwv_gi'] = _bf(np.concatenate(
            [inp['w_win_v'][:, h * VHD:(h + 1) * VHD] for h in hs]
            + [inp['w_gate'], inp['w_imp']], axis=1))
        m['Wsv'] = _bf(np.concatenate(
            [inp['w_sel_v'][:, h * VHD:(h + 1) * VHD] for h in hs], axis=1))
        m['Wproj'] = _bf(np.concatenate(
            [inp['w_proj'][h * VHD:(h + 1) * VHD, :] for h in hs], axis=1))
        maps.append(m)
    return maps


def run(inputs, **kw):
    nc = _build()
    in_maps = host_prep(inputs)
    res = bass_utils.run_bass_kernel_spmd(nc, in_maps, core_ids=list(range(N_CORES)),
                                          **kw)
    y = np.zeros((T, C), np.float32)
    for r in res.results:
        y += r['y']
    return y[None].astype(np.float32), res


def kernel(**inputs):
    y, _ = run(inputs)
    return y
